# revision 18
# baseline (speedup 1.0000x reference)
"""BasicTransformerBlock on 8 TRN2 NeuronCores.

Sharding: sequence-parallel, zero collectives. The [B=2, N=2048, D=512]
residual stream is split into 8 row-blocks of 512 (4 cores per batch
element). Every core recomputes the cheap batch-wide work it needs
(adaln1 + K/V projections over its batch's 2048 rows, cond K/V), and does
attention / FFN only for its own 512 query rows.

Per-core inputs are pre-rotated with np.roll so that "own" rows are always
rows 0:512 -- the SPMD program is identical on all cores, only data differs.
Attention is permutation-invariant over keys, so rolled K/V is fine.

Attention is computed with transposed scores sT[j, i] so exp() runs on
ScalarE straight out of PSUM. Score matmuls are made DENSE (K=128) by
stacking both heads of a head-pair in the stationary operand and zero-
padding the query operand per head: the PE_HAM activity monitor only
releases the 2.4 GHz clock for full-array matmuls, and K=64 attention
matmuls otherwise run the whole phase at 1.2 GHz. The 65th v-column of
ones makes the softmax denominator fall out of the attn@v matmul.

PSUM during attention: a 6-slot score ring (6 banks) + 2 accumulator
banks. exp is issued per 3-slot granule ([128, 3, 512] per ACTIVATE) to
amortize the 352-cycle ACT instruction overhead.
"""

import contextlib

import numpy as np

import concourse.bass as bass
import concourse.mybir as mybir
import concourse.tile as tile
from concourse import bacc
from concourse.bass_utils import run_bass_kernel_spmd
from concourse.masks import make_identity

dt = mybir.dt
AF = mybir.ActivationFunctionType
OP = mybir.AluOpType

B, N, D = 2, 2048, 512
NCTX = 1024          # cond length
H = 8                # heads
HD = D // H          # 64
EPS = 1e-5
P = 128              # partitions
NCORES = 8
ROWS = 512           # own rows per core
NB = N               # batch rows per core (2048)
SCALE = HD ** -0.5   # 0.125

f32 = dt.float32
bf16 = dt.bfloat16

_CACHED = {}


def _adaln_stats(nc, stat_pool, src_tiles, n_tiles, eps_sb, chunk=4):
    """bn_stats/aggr + rstd/nmr for n_tiles row-tiles. Returns (rstd_all, nmr_all)."""
    mv_all = stat_pool.tile([P, n_tiles, 2], f32)
    rstd_all = stat_pool.tile([P, n_tiles], f32)
    nmr_all = stat_pool.tile([P, n_tiles], f32)
    for c0 in range(0, n_tiles, chunk):
        for it in range(c0, c0 + chunk):
            stats = stat_pool.tile([P, 6], f32, tag="stats")
            nc.vector.bn_stats(stats, src_tiles(it))
            nc.vector.bn_aggr(mv_all[:, it, :], stats)
        cs = slice(c0, c0 + chunk)
        nc.scalar.activation(rstd_all[:, cs], mv_all[:, cs, 1], AF.Sqrt,
                             bias=eps_sb, scale=1.0)
        nc.vector.reciprocal(rstd_all[:, cs], rstd_all[:, cs])
        nc.vector.scalar_tensor_tensor(
            nmr_all[:, cs], mv_all[:, cs, 0], -1.0, rstd_all[:, cs],
            op0=OP.mult, op1=OP.mult,
        )
    return rstd_all, nmr_all


def _adaln_apply_tile(nc, xn_pool, pst_pool, src, it, ab, rstd_all, nmr_all,
                      hT, ident_bf16, on_act=False):
    """One tile: xn = (x-mean)*rstd -> PE transpose -> (1+scale)/shift -> hT."""
    xn = xn_pool.tile([P, 512], bf16, tag="xn")
    if on_act:
        nc.scalar.activation(xn, src, AF.Identity,
                             bias=nmr_all[:, it:it + 1],
                             scale=rstd_all[:, it:it + 1])
    else:
        nc.vector.tensor_scalar(
            xn, src, rstd_all[:, it:it + 1], nmr_all[:, it:it + 1],
            op0=OP.mult, op1=OP.add,
        )
    xnt = pst_pool.tile([P, 4, P], bf16, tag="xnt")
    for b in range(4):
        nc.tensor.transpose(xnt[:, b, :], xn[:, b * P:(b + 1) * P], ident_bf16)
    for b in range(4):
        nc.vector.tensor_scalar(
            hT[:, b, it * P:(it + 1) * P], xnt[:, b, :],
            ab[:, b:b + 1], ab[:, 4 + b:5 + b],
            op0=OP.mult, op1=OP.add,
        )


def _adaln_to_hT(nc, tc, src_tiles, n_tiles, ab, hT, ident_bf16, eps_sb, name):
    with contextlib.ExitStack() as actx:
        stat_pool = actx.enter_context(tc.tile_pool(name=f"{name}_stat", bufs=4))
        xn_pool = actx.enter_context(tc.tile_pool(name=f"{name}_xn", bufs=3))
        pst_pool = actx.enter_context(
            tc.tile_pool(name=f"{name}_pst", bufs=2, space="PSUM"))
        rstd, nmr = _adaln_stats(nc, stat_pool, src_tiles, n_tiles, eps_sb,
                                 chunk=n_tiles)
        for it in range(n_tiles):
            _adaln_apply_tile(nc, xn_pool, pst_pool, src_tiles(it), it, ab,
                              rstd, nmr, hT, ident_bf16, on_act=False)


def _attention(nc, tc, act, qpad, kT, v, njt, wo, ob_row, ones_row,
               x_res, x_out, name):
    """Dense-score attention for 8 heads (4 pairs) over own 512 rows.

    qpad: [128, 4, 2, ROWS] bf16 zero-padded per head half.
    kT:   [128, 4, njt*128] bf16 (partitions = paired head dims).
    v:    [128, njt, 8, 65] bf16 (col 64 of each head = 1.0).
    Writes x_out = attn_out @ wo + ob + x_res  (all [128, 4, 512] f32).
    """
    av_all = act.tile([P, 4, ROWS], bf16, tag="tH")
    S = 2 * njt                       # score slots per pair
    G = (S + 2) // 3                  # exp granules per pair
    with (
        tc.tile_pool(name=f"{name}_ps_s", bufs=1, space="PSUM") as ps_s,
        tc.tile_pool(name=f"{name}_ps_av", bufs=1, space="PSUM") as ps_av,
        tc.tile_pool(name=f"{name}_et", bufs=3) as et_pool,
        tc.tile_pool(name=f"{name}_dn", bufs=4) as dn_pool,
        tc.tile_pool(name=f"{name}_rb", bufs=2) as rb_pool,
    ):
        sps = ps_s.tile([P, 6, ROWS], f32)
        for ht in range(4):           # head pair (2ht, 2ht+1)
            avp = ps_av.tile([HD + 1, 2, ROWS], f32, tag="av")
            for g in range(G + 1):
                if g < G:
                    lo, hi = 3 * g, min(3 * g + 3, S)
                    for s in range(lo, hi):
                        jt, hl = s // 2, s % 2
                        nc.tensor.matmul(
                            sps[:, s % 6, :],
                            kT[:, ht, jt * P:(jt + 1) * P],
                            qpad[:, ht, hl, :],
                            start=True, stop=True,
                        )
                if g >= 1:
                    lo, hi = 3 * (g - 1), min(3 * (g - 1) + 3, S)
                    ng = hi - lo
                    et = et_pool.tile([P, 3, ROWS], bf16, tag="et")
                    lo6 = lo % 6
                    nc.scalar.activation(
                        et[:, 0:ng, :], sps[:, lo6:lo6 + ng, :], AF.Exp,
                        scale=SCALE,
                    )
                    for i, s in enumerate(range(lo, hi)):
                        jt, hl = s // 2, s % 2
                        h = 2 * ht + hl
                        nc.tensor.matmul(
                            avp[:, hl, :], v[:, jt, h, :], et[:, i, :],
                            start=(jt == 0), stop=(jt == njt - 1),
                        )
            # softmax denominators: row 64 of each accumulator. Broadcast
            # across partitions with K=1 matmuls into a retired ring slot,
            # then one fast reciprocal for the pair.
            rb_slot = (3 * G) % 6
            for hl in range(2):
                dnm = dn_pool.tile([1, ROWS], bf16, tag="dnm")
                nc.vector.tensor_copy(dnm, avp[HD:HD + 1, hl, :])
                nc.tensor.matmul(
                    sps[hl * HD:(hl + 1) * HD, rb_slot, :],
                    ones_row[0:1, 0:HD], dnm,
                    start=True, stop=True,
                )
            rb_sb = rb_pool.tile([P, ROWS], f32, tag="rb")
            nc.vector.reciprocal_approx_fast(rb_sb, sps[:, rb_slot, :])
            for hl in range(2):
                po = hl * HD
                nc.vector.scalar_tensor_tensor(
                    av_all[po:po + HD, ht, :],
                    avp[0:HD, hl, :], 1.0, rb_sb[po:po + HD, :],
                    op0=OP.mult, op1=OP.mult,
                )
    # out-projection + bias + residual
    with tc.tile_pool(name=f"{name}_ps_o", bufs=2, space="PSUM") as ps_o:
        for it in range(4):
            ps = ps_o.tile([P, D], f32, tag="o")
            for dt_ in range(4):
                nc.tensor.matmul(
                    ps, av_all[:, dt_, it * P:(it + 1) * P], wo[:, dt_, :],
                    start=(dt_ == 0), stop=False,
                )
            nc.tensor.matmul(
                ps, ones_row[0:1, 0:P], ob_row, start=False, stop=True,
            )
            nc.vector.tensor_tensor(x_out[:, it, :], ps, x_res[:, it, :], op=OP.add)


def build():
    nc = bacc.Bacc(None, target_bir_lowering=False)

    # ---------------- I/O ----------------
    xb = nc.dram_tensor("xb", [NB, D], f32, kind="ExternalInput")
    condb = nc.dram_tensor("condb", [NCTX, D], f32, kind="ExternalInput")
    t_in = nc.dram_tensor("t", [D], f32, kind="ExternalInput")
    nw = {}
    nb_ = {}
    for l in (1, 2, 4):
        nw[l] = nc.dram_tensor(f"n{l}_w", [D, 2 * D], f32, kind="ExternalInput")
        nb_[l] = nc.dram_tensor(f"n{l}_b", [2 * D], f32, kind="ExternalInput")
    aw = {}
    for a in (1, 2):
        for w in "qkvo":
            aw[a, w] = nc.dram_tensor(f"a{a}_{w}", [D, D], f32, kind="ExternalInput")
        aw[a, "ob"] = nc.dram_tensor(f"a{a}_ob", [D], f32, kind="ExternalInput")
    ff_w1 = nc.dram_tensor("ff_w1", [D, 8 * D], f32, kind="ExternalInput")
    ff_b1 = nc.dram_tensor("ff_b1", [8 * D], f32, kind="ExternalInput")
    ff_w2 = nc.dram_tensor("ff_w2", [4 * D, D], f32, kind="ExternalInput")
    ff_b2 = nc.dram_tensor("ff_b2", [D], f32, kind="ExternalInput")
    out = nc.dram_tensor("out", [ROWS, D], f32, kind="ExternalOutput")

    with tile.TileContext(nc) as tc, contextlib.ExitStack() as ctx:
        const = ctx.enter_context(tc.tile_pool(name="const", bufs=1))
        wpool = ctx.enter_context(tc.tile_pool(name="wpool", bufs=1))
        act = ctx.enter_context(tc.tile_pool(name="act", bufs=1))

        ident_bf16 = const.tile([P, P], bf16)
        make_identity(nc, ident_bf16)
        ident_f32 = const.tile([P, P], f32)
        make_identity(nc, ident_f32)
        ones_row = const.tile([1, P], bf16)
        nc.vector.memset(ones_row, 1.0)
        eps_sb = const.tile([P, 1], f32)
        nc.vector.memset(eps_sb, EPS)

        # ---------------- DMA issue order --------------------------------
        # gpsimd queue: tT, nw (small, gate the emb->ab chain), then weight
        # stacks in first-use order. sync queue: nb rows, x, b1.
        tT = const.tile([P, 4], bf16)
        nc.gpsimd.dma_start(tT, t_in[:].rearrange("(k p) -> p k", p=P))

        ab = {}
        xr_pool = ctx.enter_context(tc.tile_pool(name="xrp", bufs=6))
        cond_pool = ctx.enter_context(tc.tile_pool(name="cin", bufs=4))
        n1_stat = ctx.enter_context(tc.tile_pool(name="n1_stat", bufs=4))
        with (
            tc.tile_pool(name="nwp", bufs=2) as nwp,
            tc.tile_pool(name="embp", bufs=1) as embp,
        ):
            nw_sb = {}
            nb_row = {}
            for l in (1, 2):
                nw_sb[l] = nwp.tile([P, 4, 2 * D], bf16, tag="nw",
                                    name=f"nw_sb{l}")
                nc.gpsimd.dma_start(
                    nw_sb[l], nw[l][:].rearrange("(k p) n -> p k n", p=P)
                )

            h1T = act.tile([P, 4, NB], bf16, tag="tA")
            own_x = act.tile([P, 4, D], f32, tag="tE")
            x_tiles = {}
            for it in range(16):
                if it < 4:
                    dst = own_x[:, it, :]
                else:
                    dst = xr_pool.tile([P, D], f32, tag="xr", name=f"xr{it}")
                nc.sync.dma_start(dst, xb[:][it * P:(it + 1) * P, :])
                x_tiles[it] = dst

            for l in (1, 2, 4):
                nb_row[l] = embp.tile([1, 2 * D], f32, tag="nbrow",
                                      name=f"nb_row{l}")
                nc.sync.dma_start(nb_row[l],
                                  nb_[l][:].rearrange("(a n) -> a n", a=1))

            # attention weights (bf16 via DMA cast); a1 shares addresses with
            # ff_w1, a2 with ff_w2 (sequential lifetimes; WAR deps inserted)
            a_sb = {}
            for a, wtag, order in ((1, "wbig1", "kvqo"), (2, "wbig2", "kvqo")):
                stack = wpool.tile([P, 4, 4, D], bf16, tag=wtag)
                for wi, w in enumerate("qkvo"):
                    a_sb[a, w] = stack[:, :, wi, :]
                for w in order:
                    wi = "qkvo".index(w)
                    nc.gpsimd.dma_start(
                        stack[:, :, wi, :],
                        aw[a, w][:].rearrange("(k p) n -> p k n", p=P),
                    )
                ob = wpool.tile([1, D], bf16, tag=f"a{a}ob")
                nc.gpsimd.dma_start(ob,
                                    aw[a, "ob"][:].rearrange("(a n) -> a n", a=1))
                a_sb[a, "ob"] = ob
                if a == 1:
                    # n4_w is not needed until adaln3; park its DMA here so
                    # it reuses nw_sb[1]'s ring slot without stalling a1
                    nw_sb[4] = nwp.tile([P, 4, 2 * D], bf16, tag="nw",
                                        name="nw_sb4")
                    nc.gpsimd.dma_start(
                        nw_sb[4], nw[4][:].rearrange("(k p) n -> p k n", p=P)
                    )

            # cond, cast to bf16 on the way in (feeds bf16 PE transposes)
            c_sb = {}
            for it in range(8):
                c_sb[it] = cond_pool.tile([P, D], bf16, tag="ctile",
                                          name=f"c_sb{it}")
                nc.gpsimd.dma_start(c_sb[it], condb[:][it * P:(it + 1) * P, :])

            # PE warmup: dependency-free matmuls fill the startup DMA window
            with tc.tile_pool(name="warm", bufs=1, space="PSUM") as warm_pool:
                wps = warm_pool.tile([P, P], f32)
                for _ in range(50):
                    nc.tensor.matmul(wps, ident_bf16, ident_bf16,
                                     start=True, stop=True)

            # adaln1 stats first: independent of norm weights
            rstd1, nmr1 = _adaln_stats(nc, n1_stat, lambda it: x_tiles[it],
                                       16, eps_sb)

            # ------------- norm scale/shift params -----------------------
            with tc.tile_pool(name="ps_emb", bufs=2, space="PSUM") as ps_emb:
                for l in (1, 2, 4):
                    emb_ps = ps_emb.tile([1, 2 * D], f32, tag="embps")
                    for half in range(2):
                        for kt in range(4):
                            nc.tensor.matmul(
                                emb_ps[:, half * D:(half + 1) * D],
                                tT[:, kt:kt + 1],
                                nw_sb[l][:, kt, half * D:(half + 1) * D],
                                start=(kt == 0), stop=(kt == 3),
                            )
                    emb_row = embp.tile([1, 2 * D], f32, tag="embrow")
                    nc.vector.tensor_tensor(emb_row, emb_ps, nb_row[l], op=OP.add)
                    ab_l = const.tile([P, 8], f32, tag=f"ab{l}")
                    for col in range(8):
                        tp = ps_emb.tile([P, 1], f32, tag="embT")
                        nc.tensor.transpose(
                            tp, emb_row[0:1, col * P:(col + 1) * P],
                            ident_f32[0:1, 0:1]
                        )
                        nc.vector.tensor_scalar(
                            ab_l[:, col:col + 1], tp,
                            1.0 if col < 4 else 0.0, None, op0=OP.add,
                        )
                    ab[l] = ab_l

        # ---------------- adaln1 apply + projections, interleaved --------
        # Per 512-row block: xn (ACT; free this early) -> PE transpose ->
        # affine -> h1T cols, then immediately the dense k1/v1 (and q1)
        # matmuls for that block so the PE never starves.
        k1T = act.tile([P, 4, NB], bf16, tag="tB")
        v1 = act.tile([P, 16, H, HD + 1], bf16, tag="tC")
        q1pad = act.tile([P, 4, 2, ROWS], bf16, tag="tD")
        nc.vector.memset(v1[:, :, :, HD:HD + 1], 1.0)
        nc.vector.memset(q1pad, 0.0)
        with (
            tc.tile_pool(name="n1_xn", bufs=3) as xn_pool,
            tc.tile_pool(name="n1_pst", bufs=2, space="PSUM") as pst_pool,
            tc.tile_pool(name="ps_proj1", bufs=4, space="PSUM") as ps_proj,
        ):
            for jc in range(4):
                for it in range(4 * jc, 4 * jc + 4):
                    _adaln_apply_tile(nc, xn_pool, pst_pool, x_tiles[it], it,
                                      ab[1], rstd1, nmr1, h1T, ident_bf16,
                                      on_act=True)
                for dt_ in range(4):
                    ps = ps_proj.tile([P, 512], f32, tag="proj")
                    for kt in range(4):
                        nc.tensor.matmul(
                            ps,
                            a_sb[1, "k"][:, kt, dt_ * P:(dt_ + 1) * P],
                            h1T[:, kt, jc * 512:(jc + 1) * 512],
                            start=(kt == 0), stop=(kt == 3),
                        )
                    nc.vector.tensor_copy(
                        k1T[:, dt_, jc * 512:(jc + 1) * 512], ps
                    )
                for jt in range(4 * jc, 4 * jc + 4):
                    ps = ps_proj.tile([P, 512], f32, tag="proj")
                    for kt in range(4):
                        nc.tensor.matmul(
                            ps,
                            h1T[:, kt, jt * P:(jt + 1) * P],
                            a_sb[1, "v"][:, kt, :],
                            start=(kt == 0), stop=(kt == 3),
                        )
                    nc.vector.tensor_copy(
                        v1[:, jt, :, 0:HD], ps.rearrange("p (h d) -> p h d", h=H)
                    )
                if jc == 0:
                    for dt_ in range(4):
                        ps = ps_proj.tile([P, 512], f32, tag="proj")
                        for kt in range(4):
                            nc.tensor.matmul(
                                ps,
                                a_sb[1, "q"][:, kt, dt_ * P:(dt_ + 1) * P],
                                h1T[:, kt, 0:ROWS],
                                start=(kt == 0), stop=(kt == 3),
                            )
                        nc.vector.tensor_copy(q1pad[0:HD, dt_, 0, :], ps[0:HD, :])
                        nc.vector.tensor_copy(q1pad[HD:P, dt_, 1, :], ps[HD:P, :])

        # ------- cond transposes (cheap, fills PE before attn1) ----------
        # condT + k2T live in the retired xrest region (tag tX)
        ctk2 = act.tile([P, 2, 4, NCTX], bf16, tag="tX")
        condT = ctk2[:, 0, :, :]
        k2T = ctk2[:, 1, :, :]
        with tc.tile_pool(name="ps_ct", bufs=2, space="PSUM") as ps_ct:
            for it in range(8):
                ct = ps_ct.tile([P, 4, P], bf16, tag="ct")
                for b in range(4):
                    nc.tensor.transpose(
                        ct[:, b, :], c_sb[it][:, b * P:(b + 1) * P], ident_bf16
                    )
                for b in range(4):
                    nc.vector.tensor_copy(
                        condT[:, b, it * P:(it + 1) * P], ct[:, b, :]
                    )

        # ---------------- attention 1 ------------------------------------
        x2 = act.tile([P, 4, D], f32, tag="tF")
        _attention(nc, tc, act, q1pad, k1T, v1, 16, a_sb[1, "o"],
                   a_sb[1, "ob"], ones_row, own_x, x2, "att1")

        # ------- cross-attn prep: k2T, v2 (independent of x) -------------
        v2 = act.tile([P, 8, H, HD + 1], bf16, tag="tI")
        nc.vector.memset(v2[:, :, :, HD:HD + 1], 1.0)
        with tc.tile_pool(name="ps_proj2a", bufs=4, space="PSUM") as ps_proj:
            for dt_ in range(4):
                for jc in range(2):
                    ps = ps_proj.tile([P, 512], f32, tag="proj")
                    for kt in range(4):
                        nc.tensor.matmul(
                            ps,
                            a_sb[2, "k"][:, kt, dt_ * P:(dt_ + 1) * P],
                            condT[:, kt, jc * 512:(jc + 1) * 512],
                            start=(kt == 0), stop=(kt == 3),
                        )
                    nc.vector.tensor_copy(
                        k2T[:, dt_, jc * 512:(jc + 1) * 512], ps
                    )
            for jt in range(8):
                ps = ps_proj.tile([P, 512], f32, tag="proj")
                for kt in range(4):
                    nc.tensor.matmul(
                        ps,
                        condT[:, kt, jt * P:(jt + 1) * P],
                        a_sb[2, "v"][:, kt, :],
                        start=(kt == 0), stop=(kt == 3),
                    )
                nc.vector.tensor_copy(
                    v2[:, jt, :, 0:HD], ps.rearrange("p (h d) -> p h d", h=H)
                )

        # ---------------- adaln2 + cross-attn ----------------------------
        h2T = act.tile([P, 4, ROWS], bf16, tag="tH")
        _adaln_to_hT(nc, tc, lambda it: x2[:, it, :], 4, ab[2], h2T,
                     ident_bf16, eps_sb, "n2")

        q2pad = act.tile([P, 4, 2, ROWS], bf16, tag="tD")
        nc.vector.memset(q2pad, 0.0)
        with tc.tile_pool(name="ps_proj2b", bufs=2, space="PSUM") as ps_proj:
            for dt_ in range(4):
                ps = ps_proj.tile([P, 512], f32, tag="proj")
                for kt in range(4):
                    nc.tensor.matmul(
                        ps,
                        a_sb[2, "q"][:, kt, dt_ * P:(dt_ + 1) * P],
                        h2T[:, kt, :],
                        start=(kt == 0), stop=(kt == 3),
                    )
                nc.vector.tensor_copy(q2pad[0:HD, dt_, 0, :], ps[0:HD, :])
                nc.vector.tensor_copy(q2pad[HD:P, dt_, 1, :], ps[HD:P, :])

        x3 = act.tile([P, 4, D], f32, tag="tG")
        _attention(nc, tc, act, q2pad, k2T, v2, 8, a_sb[2, "o"],
                   a_sb[2, "ob"], ones_row, x2, x3, "att2")

        # FFN weights: issue after the a2 stack is consumed (shared tags)
        w1_sb = wpool.tile([P, 4, 8 * D], bf16, tag="wbig1")
        nc.gpsimd.dma_start(w1_sb, ff_w1[:].rearrange("(k p) n -> p k n", p=P))
        w2_sb = wpool.tile([P, 16, D], bf16, tag="wbig2")
        nc.gpsimd.dma_start(w2_sb, ff_w2[:].rearrange("(k p) n -> p k n", p=P))
        b1_sb = const.tile([P, 32], f32)
        nc.sync.dma_start(b1_sb, ff_b1[:].rearrange("(k p) -> p k", p=P))
        b2_row = const.tile([1, D], bf16)
        nc.gpsimd.dma_start(b2_row, ff_b2[:].rearrange("(a n) -> a n", a=1))

        # ---------------- adaln3 + GEGLU FFN -----------------------------
        h3T = act.tile([P, 4, ROWS], bf16, tag="tJ")
        _adaln_to_hT(nc, tc, lambda it: x3[:, it, :], 4, ab[4], h3T,
                     ident_bf16, eps_sb, "n4")

        # per-ut pipeline: zu/zg -> gelu/stt -> 4 y-accumulator matmuls.
        # y accumulates in 4 persistent PSUM banks across all 16 ut chunks.
        ugT = act.tile([P, 16, ROWS], bf16, tag="tA")
        out_sb = act.tile([P, 4, D], f32, tag="tC")
        with (
            tc.tile_pool(name="ps_z", bufs=4, space="PSUM") as ps_z,
            tc.tile_pool(name="ps_y", bufs=1, space="PSUM") as ps_y,
            tc.tile_pool(name="gact", bufs=3) as gact_pool,
        ):
            y_ps = ps_y.tile([P, 4, D], f32)
            for ut in range(16):
                zu = ps_z.tile([P, ROWS], f32, tag="z")
                zg = ps_z.tile([P, ROWS], f32, tag="z")
                for kt in range(4):
                    nc.tensor.matmul(
                        zu, w1_sb[:, kt, ut * P:(ut + 1) * P],
                        h3T[:, kt, :], start=(kt == 0), stop=(kt == 3),
                    )
                for kt in range(4):
                    nc.tensor.matmul(
                        zg, w1_sb[:, kt, (16 + ut) * P:(17 + ut) * P],
                        h3T[:, kt, :], start=(kt == 0), stop=(kt == 3),
                    )
                gact = gact_pool.tile([P, ROWS], bf16, tag="gact")
                nc.scalar.activation(
                    gact, zg, AF.Gelu, bias=b1_sb[:, 16 + ut:17 + ut], scale=1.0
                )
                nc.vector.scalar_tensor_tensor(
                    ugT[:, ut, :], zu, b1_sb[:, ut:ut + 1], gact,
                    op0=OP.add, op1=OP.mult,
                )
                for it in range(4):
                    nc.tensor.matmul(
                        y_ps[:, it, :], ugT[:, ut, it * P:(it + 1) * P],
                        w2_sb[:, ut, :],
                        start=(ut == 0), stop=False,
                    )
            for it in range(4):
                nc.tensor.matmul(
                    y_ps[:, it, :], ones_row[0:1, 0:P], b2_row,
                    start=False, stop=True,
                )
                nc.vector.tensor_tensor(
                    out_sb[:, it, :], y_ps[:, it, :], x3[:, it, :], op=OP.add
                )

        for it_ in range(4):
            nc.sync.dma_start(out[:][it_ * P:(it_ + 1) * P, :], out_sb[:, it_, :])

    nc.compile()
    return nc


def _shard_inputs(inputs):
    """Build the 8 per-core input maps."""
    x = np.ascontiguousarray(inputs["x"], dtype=np.float32)
    t = np.ascontiguousarray(inputs["t"], dtype=np.float32)
    cond = np.ascontiguousarray(inputs["cond"], dtype=np.float32)
    shared = {}
    for k in ("n1_w", "n1_b", "n2_w", "n2_b", "n4_w", "n4_b",
              "a1_q", "a1_k", "a1_v", "a1_o", "a1_ob",
              "a2_q", "a2_k", "a2_v", "a2_o", "a2_ob",
              "ff_w1", "ff_b1", "ff_w2", "ff_b2"):
        shared[k] = np.ascontiguousarray(inputs[k], dtype=np.float32)
    in_maps = []
    for c in range(NCORES):
        b = c // 4
        r0 = (c % 4) * ROWS
        m = dict(shared)
        m["xb"] = np.ascontiguousarray(np.roll(x[b], -r0, axis=0))
        m["condb"] = np.ascontiguousarray(cond[b])
        m["t"] = np.ascontiguousarray(t[b, 0])
        in_maps.append(m)
    return in_maps


def kernel(**inputs) -> np.ndarray:
    if "nc" not in _CACHED:
        _CACHED["nc"] = build()
    nc = _CACHED["nc"]
    in_maps = _shard_inputs(inputs)
    res = run_bass_kernel_spmd(nc, in_maps, core_ids=list(range(NCORES)))
    outs = [res.results[c]["out"] for c in range(NCORES)]
    full = np.concatenate(outs, axis=0).reshape(B, N, D)
    return full.astype(np.float32)


# revision 21
# speedup vs baseline: 1.1922x; 1.1922x over previous
"""BasicTransformerBlock on 8 TRN2 NeuronCores.

Sharding: sequence-parallel, zero collectives. The [B=2, N=2048, D=512]
residual stream is split into 8 row-blocks of 512 (4 cores per batch
element). Every core recomputes the cheap batch-wide work it needs
(adaln1 + K/V projections over its batch's 2048 rows, cond K/V), and does
attention / FFN only for its own 512 query rows.

Host-side prep (part of kernel()'s sharding layer, shared across cores):
weights are pre-cast to bf16 and pre-rearranged into the [partition, ktile,
cols] SBUF layouts, and cond is pre-transposed, so every DMA is a fast
contiguous HWDGE transfer (no software-DGE casting scatter loads) and no
on-chip cond transposes are needed. x stays f32 (layernorm stats need it).
Per-core x is pre-rotated with np.roll so "own" rows are always rows 0:512;
attention is permutation-invariant over keys, so rolled K/V is fine.

Attention: transposed scores sT[j, i] so exp() runs on ScalarE straight out
of PSUM. Score matmuls are DENSE (K=128): both heads of a pair are stacked
in the stationary operand and the query operand is zero-padded per head.
The PE_HAM activity monitor only grants the 2.4 GHz clock to full-array
matmuls; K=64 attention matmuls would run the whole phase at 1.2 GHz.
The 65th v-column of ones makes the softmax denominator fall out of the
attn@v matmul.

PSUM during attention: a 6-slot score ring (6 banks) + 2 accumulator
banks. exp is issued per 3-slot granule ([128, 3, 512] per ACTIVATE) to
amortize ACT instruction overhead, and the loop is skewed exp(g-1) /
scores(g) / av(g-2) so the PE batch for step g only depends on ACT work
from step g-2 -- the engines overlap instead of ping-ponging.
"""

import contextlib

import ml_dtypes
import numpy as np

import concourse.bass as bass
import concourse.mybir as mybir
import concourse.tile as tile
from concourse import bacc
from concourse.bass_utils import run_bass_kernel_spmd
from concourse.masks import make_identity

dt = mybir.dt
AF = mybir.ActivationFunctionType
OP = mybir.AluOpType

B, N, D = 2, 2048, 512
NCTX = 1024          # cond length
H = 8                # heads
HD = D // H          # 64
EPS = 1e-5
P = 128              # partitions
NCORES = 8
ROWS = 512           # own rows per core
NB = N               # batch rows per core (2048)
SCALE = HD ** -0.5   # 0.125

f32 = dt.float32
bf16 = dt.bfloat16

_CACHED = {}


def _adaln_stats(nc, stat_pool, src_tiles, n_tiles, eps_sb, chunk=4):
    """bn_stats/aggr + rstd/nmr for n_tiles row-tiles. Returns (rstd_all, nmr_all)."""
    mv_all = stat_pool.tile([P, n_tiles, 2], f32)
    rstd_all = stat_pool.tile([P, n_tiles], f32)
    nmr_all = stat_pool.tile([P, n_tiles], f32)
    for c0 in range(0, n_tiles, chunk):
        for it in range(c0, c0 + chunk):
            stats = stat_pool.tile([P, 6], f32, tag="stats")
            nc.vector.bn_stats(stats, src_tiles(it))
            nc.vector.bn_aggr(mv_all[:, it, :], stats)
        cs = slice(c0, c0 + chunk)
        nc.scalar.activation(rstd_all[:, cs], mv_all[:, cs, 1], AF.Sqrt,
                             bias=eps_sb, scale=1.0)
        nc.vector.reciprocal(rstd_all[:, cs], rstd_all[:, cs])
        nc.vector.scalar_tensor_tensor(
            nmr_all[:, cs], mv_all[:, cs, 0], -1.0, rstd_all[:, cs],
            op0=OP.mult, op1=OP.mult,
        )
    return rstd_all, nmr_all


def _adaln_apply_tile(nc, xn_pool, pst_pool, src, it, ab, rstd_all, nmr_all,
                      hT, ident_bf16, on_act=False):
    """One tile: xn = (x-mean)*rstd -> PE transpose -> (1+scale)/shift -> hT."""
    xn = xn_pool.tile([P, 512], bf16, tag="xn")
    if on_act:
        nc.scalar.activation(xn, src, AF.Identity,
                             bias=nmr_all[:, it:it + 1],
                             scale=rstd_all[:, it:it + 1])
    else:
        nc.vector.tensor_scalar(
            xn, src, rstd_all[:, it:it + 1], nmr_all[:, it:it + 1],
            op0=OP.mult, op1=OP.add,
        )
    xnt = pst_pool.tile([P, 4, P], bf16, tag="xnt")
    for b in range(4):
        nc.tensor.transpose(xnt[:, b, :], xn[:, b * P:(b + 1) * P], ident_bf16)
    for b in range(4):
        nc.vector.tensor_scalar(
            hT[:, b, it * P:(it + 1) * P], xnt[:, b, :],
            ab[:, b:b + 1], ab[:, 4 + b:5 + b],
            op0=OP.mult, op1=OP.add,
        )


def _adaln_to_hT(nc, tc, src_tiles, n_tiles, ab, hT, ident_bf16, eps_sb, name):
    with contextlib.ExitStack() as actx:
        stat_pool = actx.enter_context(tc.tile_pool(name=f"{name}_stat", bufs=4))
        xn_pool = actx.enter_context(tc.tile_pool(name=f"{name}_xn", bufs=3))
        pst_pool = actx.enter_context(
            tc.tile_pool(name=f"{name}_pst", bufs=2, space="PSUM"))
        rstd, nmr = _adaln_stats(nc, stat_pool, src_tiles, n_tiles, eps_sb,
                                 chunk=n_tiles)
        for it in range(n_tiles):
            _adaln_apply_tile(nc, xn_pool, pst_pool, src_tiles(it), it, ab,
                              rstd, nmr, hT, ident_bf16, on_act=False)


def _attention(nc, tc, act, qpad, kT, v, njt, wo, ob_row, ones_row,
               x_res, x_out, name):
    """Dense-score attention for 8 heads (4 pairs) over own 512 rows.

    qpad: [128, 4, 2, ROWS] bf16 zero-padded per head half.
    kT:   [128, 4, njt*128] bf16 (partitions = paired head dims).
    v:    [128, njt, 8, 65] bf16 (col 64 of each head = 1.0).
    Writes x_out = attn_out @ wo + ob + x_res  (all [128, 4, 512] f32).
    """
    av_all = act.tile([P, 4, ROWS], bf16, tag="tH")
    S = 2 * njt                       # score slots per pair
    G = (S + 2) // 3                  # exp granules per pair
    with (
        tc.tile_pool(name=f"{name}_ps_s", bufs=1, space="PSUM") as ps_s,
        tc.tile_pool(name=f"{name}_ps_av", bufs=1, space="PSUM") as ps_av,
        tc.tile_pool(name=f"{name}_et", bufs=3) as et_pool,
        tc.tile_pool(name=f"{name}_dn", bufs=4) as dn_pool,
        tc.tile_pool(name=f"{name}_rb", bufs=2) as rb_pool,
    ):
        sps = ps_s.tile([P, 6, ROWS], f32)
        for ht in range(4):           # head pair (2ht, 2ht+1)
            avp = ps_av.tile([HD + 1, 2, ROWS], f32, tag="av")
            ets = {}
            for g in range(G + 2):
                if 1 <= g <= G:
                    lo, hi = 3 * (g - 1), min(3 * (g - 1) + 3, S)
                    ng = hi - lo
                    et = et_pool.tile([P, 3, ROWS], bf16, tag="et")
                    ets[g - 1] = (et, lo, hi)
                    lo6 = lo % 6
                    nc.scalar.activation(
                        et[:, 0:ng, :], sps[:, lo6:lo6 + ng, :], AF.Exp,
                        scale=SCALE,
                    )
                if g < G:
                    lo, hi = 3 * g, min(3 * g + 3, S)
                    for s in range(lo, hi):
                        jt, hl = s // 2, s % 2
                        nc.tensor.matmul(
                            sps[:, s % 6, :],
                            kT[:, ht, jt * P:(jt + 1) * P],
                            qpad[:, ht, hl, :],
                            start=True, stop=True,
                        )
                if g >= 2:
                    et, lo, hi = ets.pop(g - 2)
                    for i, s in enumerate(range(lo, hi)):
                        jt, hl = s // 2, s % 2
                        nc.tensor.matmul(
                            avp[:, hl, :], v[:, jt, 2 * ht + hl, :],
                            et[:, i, :],
                            start=(jt == 0), stop=(jt == njt - 1),
                        )
            # softmax denominators: row 64 of each accumulator. Broadcast
            # across partitions with K=1 matmuls into a retired ring slot,
            # then one fast reciprocal for the pair.
            rb_slot = (3 * G) % 6
            for hl in range(2):
                dnm = dn_pool.tile([1, ROWS], bf16, tag="dnm")
                nc.vector.tensor_copy(dnm, avp[HD:HD + 1, hl, :])
                nc.tensor.matmul(
                    sps[hl * HD:(hl + 1) * HD, rb_slot, :],
                    ones_row[0:1, 0:HD], dnm,
                    start=True, stop=True,
                )
            rb_sb = rb_pool.tile([P, ROWS], f32, tag="rb")
            nc.vector.reciprocal_approx_fast(rb_sb, sps[:, rb_slot, :])
            for hl in range(2):
                po = hl * HD
                nc.vector.scalar_tensor_tensor(
                    av_all[po:po + HD, ht, :],
                    avp[0:HD, hl, :], 1.0, rb_sb[po:po + HD, :],
                    op0=OP.mult, op1=OP.mult,
                )
    # out-projection + bias + residual
    with tc.tile_pool(name=f"{name}_ps_o", bufs=2, space="PSUM") as ps_o:
        for it in range(4):
            ps = ps_o.tile([P, D], f32, tag="o")
            for dt_ in range(4):
                nc.tensor.matmul(
                    ps, av_all[:, dt_, it * P:(it + 1) * P], wo[:, dt_, :],
                    start=(dt_ == 0), stop=False,
                )
            nc.tensor.matmul(
                ps, ones_row[0:1, 0:P], ob_row, start=False, stop=True,
            )
            nc.vector.tensor_tensor(x_out[:, it, :], ps, x_res[:, it, :], op=OP.add)


def build():
    nc = bacc.Bacc(None, target_bir_lowering=False)

    # -------- I/O (weights arrive pre-cast/pre-arranged from the host) ----
    xb = nc.dram_tensor("xb", [NB, D], f32, kind="ExternalInput")
    condT_d = nc.dram_tensor("condT", [P, 4, NCTX], bf16, kind="ExternalInput")
    tT_d = nc.dram_tensor("tT", [P, 4], bf16, kind="ExternalInput")
    nw_d = {}
    nb_d = {}
    for l in (1, 2, 4):
        nw_d[l] = nc.dram_tensor(f"nw{l}", [P, 4, 2 * D], bf16,
                                 kind="ExternalInput")
        nb_d[l] = nc.dram_tensor(f"nb{l}", [2 * D], f32, kind="ExternalInput")
    as_d = {}
    aob_d = {}
    for a in (1, 2):
        as_d[a] = nc.dram_tensor(f"a{a}s", [P, 4, 4, D], bf16,
                                 kind="ExternalInput")
        aob_d[a] = nc.dram_tensor(f"a{a}ob", [D], bf16, kind="ExternalInput")
    ffw1_d = nc.dram_tensor("ffw1", [P, 4, 8 * D], bf16, kind="ExternalInput")
    ffw2_d = nc.dram_tensor("ffw2", [P, 16, D], bf16, kind="ExternalInput")
    ffb1_d = nc.dram_tensor("ffb1", [P, 32], f32, kind="ExternalInput")
    ffb2_d = nc.dram_tensor("ffb2", [D], bf16, kind="ExternalInput")
    out = nc.dram_tensor("out", [ROWS, D], f32, kind="ExternalOutput")

    with tile.TileContext(nc) as tc, contextlib.ExitStack() as ctx:
        const = ctx.enter_context(tc.tile_pool(name="const", bufs=1))
        wpool = ctx.enter_context(tc.tile_pool(name="wpool", bufs=1))
        act = ctx.enter_context(tc.tile_pool(name="act", bufs=1))
        xr_pool = ctx.enter_context(tc.tile_pool(name="xrp", bufs=6))
        n1_stat = ctx.enter_context(tc.tile_pool(name="n1_stat", bufs=4))

        ident_bf16 = const.tile([P, P], bf16)
        make_identity(nc, ident_bf16)
        ident_f32 = const.tile([P, P], f32)
        make_identity(nc, ident_f32)
        ones_row = const.tile([1, P], bf16)
        nc.vector.memset(ones_row, 1.0)
        eps_sb = const.tile([P, 1], f32)
        nc.vector.memset(eps_sb, EPS)

        # ---------------- DMA issue order --------------------------------
        # qACT: weights in first-use order.  qSP: x, small rows, ff tail.
        tT = const.tile([P, 4], bf16)
        nc.scalar.dma_start(tT, tT_d[:])
        ab = {}
        with (
            tc.tile_pool(name="nwp", bufs=1) as nwp,
            tc.tile_pool(name="embp", bufs=1) as embp,
        ):
            nw_sb = {}
            for l in (1, 2):
                nw_sb[l] = nwp.tile([P, 4, 2 * D], bf16, tag=f"nw{l}",
                                    name=f"nw_sb{l}")
                nc.scalar.dma_start(nw_sb[l], nw_d[l][:])

            a_sb = {}
            stacks = {}
            for a, wtag in ((1, "wbig1"), (2, "wbig2")):
                stack = wpool.tile([P, 4, 4, D], bf16, tag=wtag,
                                   name=f"a{a}stk")
                stacks[a] = stack
                for wi, w in enumerate("qkvo"):
                    a_sb[a, w] = stack[:, :, wi, :]
            nc.scalar.dma_start(stacks[1], as_d[1][:])
            for a in (1, 2):
                ob = wpool.tile([1, D], bf16, tag=f"a{a}ob", name=f"a{a}ob_sb")
                nc.scalar.dma_start(ob,
                                    aob_d[a][:].rearrange("(a n) -> a n", a=1))
                a_sb[a, "ob"] = ob

            h1T = act.tile([P, 4, NB], bf16, tag="tA")
            own_x = act.tile([P, 4, D], f32, tag="tE")
            x_tiles = {}
            for it in range(16):
                if it < 4:
                    dst = own_x[:, it, :]
                else:
                    dst = xr_pool.tile([P, D], f32, tag="xr", name=f"xr{it}")
                nc.sync.dma_start(dst, xb[:][it * P:(it + 1) * P, :])
                x_tiles[it] = dst

            # condT + k2T share the tX region
            ctk2 = act.tile([P, 2, 4, NCTX], bf16, tag="tX")
            condT = ctk2[:, 0, :, :]
            k2T = ctk2[:, 1, :, :]
            nc.scalar.dma_start(condT, condT_d[:])
            nc.scalar.dma_start(stacks[2], as_d[2][:])
            nw_sb[4] = nwp.tile([P, 4, 2 * D], bf16, tag="nw1", name="nw_sb4")
            nc.scalar.dma_start(nw_sb[4], nw_d[4][:])

            nb_row = {}
            for l in (1, 2, 4):
                nb_row[l] = embp.tile([1, 2 * D], f32, tag="nbrow",
                                      name=f"nb_row{l}")
                nc.sync.dma_start(nb_row[l],
                                  nb_d[l][:].rearrange("(a n) -> a n", a=1))
            b1_sb = const.tile([P, 32], f32)
            nc.sync.dma_start(b1_sb, ffb1_d[:])
            b2_row = const.tile([1, D], bf16)
            nc.sync.dma_start(b2_row, ffb2_d[:].rearrange("(a n) -> a n", a=1))
            # ff weights on qSP after x: landed long before the FFN needs
            # them, but the buffers alias a1s/a2s so they wait on attention.
            w1_sb = wpool.tile([P, 4, 8 * D], bf16, tag="wbig1")
            nc.sync.dma_start(w1_sb, ffw1_d[:])
            w2_sb = wpool.tile([P, 16, D], bf16, tag="wbig2")
            nc.sync.dma_start(w2_sb, ffw2_d[:])

            # PE warmup: dependency-free matmuls fill the startup DMA window
            with tc.tile_pool(name="warm", bufs=1, space="PSUM") as warm_pool:
                wps = warm_pool.tile([P, P], f32)
                for _ in range(30):
                    nc.tensor.matmul(wps, ident_bf16, ident_bf16,
                                     start=True, stop=True)

            # adaln1 stats first: independent of norm weights
            rstd1, nmr1 = _adaln_stats(nc, n1_stat, lambda it: x_tiles[it],
                                       16, eps_sb)

            # ------------- norm scale/shift params -----------------------
            with tc.tile_pool(name="ps_emb", bufs=2, space="PSUM") as ps_emb:
                for l in (1, 2, 4):
                    emb_ps = ps_emb.tile([1, 2 * D], f32, tag="embps")
                    for half in range(2):
                        for kt in range(4):
                            nc.tensor.matmul(
                                emb_ps[:, half * D:(half + 1) * D],
                                tT[:, kt:kt + 1],
                                nw_sb[l][:, kt, half * D:(half + 1) * D],
                                start=(kt == 0), stop=(kt == 3),
                            )
                    emb_row = embp.tile([1, 2 * D], f32, tag="embrow",
                                        name=f"emb_row{l}")
                    nc.vector.tensor_tensor(emb_row, emb_ps, nb_row[l],
                                            op=OP.add)
                    ab_l = const.tile([P, 8], f32, tag=f"ab{l}")
                    for col in range(8):
                        tp = ps_emb.tile([P, 1], f32, tag="embT")
                        nc.tensor.transpose(
                            tp, emb_row[0:1, col * P:(col + 1) * P],
                            ident_f32[0:1, 0:1]
                        )
                        nc.vector.tensor_scalar(
                            ab_l[:, col:col + 1], tp,
                            1.0 if col < 4 else 0.0, None, op0=OP.add,
                        )
                    ab[l] = ab_l

        # ---------------- adaln1 apply + projections, interleaved --------
        k1T = act.tile([P, 4, NB], bf16, tag="tB")
        v1 = act.tile([P, 16, H, HD + 1], bf16, tag="tC")
        q1pad = act.tile([P, 4, 2, ROWS], bf16, tag="tD")
        nc.gpsimd.memset(v1[:, :, :, HD:HD + 1], 1.0)
        nc.gpsimd.memset(q1pad, 0.0)
        with (
            tc.tile_pool(name="n1_xn", bufs=3) as xn_pool,
            tc.tile_pool(name="n1_pst", bufs=2, space="PSUM") as pst_pool,
            tc.tile_pool(name="ps_proj1", bufs=4, space="PSUM") as ps_proj,
        ):
            for jc in range(4):
                for it in range(4 * jc, 4 * jc + 4):
                    _adaln_apply_tile(nc, xn_pool, pst_pool, x_tiles[it], it,
                                      ab[1], rstd1, nmr1, h1T, ident_bf16,
                                      on_act=True)
                for dt_ in range(4):
                    ps = ps_proj.tile([P, 512], f32, tag="proj")
                    for kt in range(4):
                        nc.tensor.matmul(
                            ps,
                            a_sb[1, "k"][:, kt, dt_ * P:(dt_ + 1) * P],
                            h1T[:, kt, jc * 512:(jc + 1) * 512],
                            start=(kt == 0), stop=(kt == 3),
                        )
                    nc.vector.tensor_copy(
                        k1T[:, dt_, jc * 512:(jc + 1) * 512], ps
                    )
                for jt in range(4 * jc, 4 * jc + 4):
                    ps = ps_proj.tile([P, 512], f32, tag="proj")
                    for kt in range(4):
                        nc.tensor.matmul(
                            ps,
                            h1T[:, kt, jt * P:(jt + 1) * P],
                            a_sb[1, "v"][:, kt, :],
                            start=(kt == 0), stop=(kt == 3),
                        )
                    nc.vector.tensor_copy(
                        v1[:, jt, :, 0:HD], ps.rearrange("p (h d) -> p h d", h=H)
                    )
                if jc == 0:
                    for dt_ in range(4):
                        ps = ps_proj.tile([P, 512], f32, tag="proj")
                        for kt in range(4):
                            nc.tensor.matmul(
                                ps,
                                a_sb[1, "q"][:, kt, dt_ * P:(dt_ + 1) * P],
                                h1T[:, kt, 0:ROWS],
                                start=(kt == 0), stop=(kt == 3),
                            )
                        nc.vector.tensor_copy(q1pad[0:HD, dt_, 0, :], ps[0:HD, :])
                        nc.vector.tensor_copy(q1pad[HD:P, dt_, 1, :], ps[HD:P, :])

        # ---------------- attention 1 ------------------------------------
        x2 = act.tile([P, 4, D], f32, tag="tF")
        _attention(nc, tc, act, q1pad, k1T, v1, 16, a_sb[1, "o"],
                   a_sb[1, "ob"], ones_row, own_x, x2, "att1")

        # ------- cross-attn prep: k2T, v2 (independent of x) -------------
        v2 = act.tile([P, 8, H, HD + 1], bf16, tag="tI")
        nc.gpsimd.memset(v2[:, :, :, HD:HD + 1], 1.0)
        with tc.tile_pool(name="ps_proj2a", bufs=4, space="PSUM") as ps_proj:
            for dt_ in range(4):
                for jc in range(2):
                    ps = ps_proj.tile([P, 512], f32, tag="proj")
                    for kt in range(4):
                        nc.tensor.matmul(
                            ps,
                            a_sb[2, "k"][:, kt, dt_ * P:(dt_ + 1) * P],
                            condT[:, kt, jc * 512:(jc + 1) * 512],
                            start=(kt == 0), stop=(kt == 3),
                        )
                    nc.vector.tensor_copy(
                        k2T[:, dt_, jc * 512:(jc + 1) * 512], ps
                    )
            for jt in range(8):
                ps = ps_proj.tile([P, 512], f32, tag="proj")
                for kt in range(4):
                    nc.tensor.matmul(
                        ps,
                        condT[:, kt, jt * P:(jt + 1) * P],
                        a_sb[2, "v"][:, kt, :],
                        start=(kt == 0), stop=(kt == 3),
                    )
                nc.vector.tensor_copy(
                    v2[:, jt, :, 0:HD], ps.rearrange("p (h d) -> p h d", h=H)
                )

        # ---------------- adaln2 + cross-attn ----------------------------
        h2T = act.tile([P, 4, ROWS], bf16, tag="tH")
        _adaln_to_hT(nc, tc, lambda it: x2[:, it, :], 4, ab[2], h2T,
                     ident_bf16, eps_sb, "n2")

        q2pad = act.tile([P, 4, 2, ROWS], bf16, tag="tD")
        nc.gpsimd.memset(q2pad, 0.0)
        with tc.tile_pool(name="ps_proj2b", bufs=2, space="PSUM") as ps_proj:
            for dt_ in range(4):
                ps = ps_proj.tile([P, 512], f32, tag="proj")
                for kt in range(4):
                    nc.tensor.matmul(
                        ps,
                        a_sb[2, "q"][:, kt, dt_ * P:(dt_ + 1) * P],
                        h2T[:, kt, :],
                        start=(kt == 0), stop=(kt == 3),
                    )
                nc.vector.tensor_copy(q2pad[0:HD, dt_, 0, :], ps[0:HD, :])
                nc.vector.tensor_copy(q2pad[HD:P, dt_, 1, :], ps[HD:P, :])

        x3 = act.tile([P, 4, D], f32, tag="tG")
        _attention(nc, tc, act, q2pad, k2T, v2, 8, a_sb[2, "o"],
                   a_sb[2, "ob"], ones_row, x2, x3, "att2")

        # ---------------- adaln3 + GEGLU FFN -----------------------------
        h3T = act.tile([P, 4, ROWS], bf16, tag="tJ")
        _adaln_to_hT(nc, tc, lambda it: x3[:, it, :], 4, ab[4], h3T,
                     ident_bf16, eps_sb, "n4")

        # per-ut pipeline: zu/zg -> gelu/stt -> 4 y-accumulator matmuls.
        # y accumulates in 4 persistent PSUM banks across all 16 ut chunks.
        ugT = act.tile([P, 16, ROWS], bf16, tag="tA")
        out_sb = act.tile([P, 4, D], f32, tag="tC")
        with (
            tc.tile_pool(name="ps_z", bufs=4, space="PSUM") as ps_z,
            tc.tile_pool(name="ps_y", bufs=1, space="PSUM") as ps_y,
            tc.tile_pool(name="gact", bufs=3) as gact_pool,
        ):
            y_ps = ps_y.tile([P, 4, D], f32)
            for ut in range(16):
                zu = ps_z.tile([P, ROWS], f32, tag="z")
                zg = ps_z.tile([P, ROWS], f32, tag="z")
                for kt in range(4):
                    nc.tensor.matmul(
                        zu, w1_sb[:, kt, ut * P:(ut + 1) * P],
                        h3T[:, kt, :], start=(kt == 0), stop=(kt == 3),
                    )
                for kt in range(4):
                    nc.tensor.matmul(
                        zg, w1_sb[:, kt, (16 + ut) * P:(17 + ut) * P],
                        h3T[:, kt, :], start=(kt == 0), stop=(kt == 3),
                    )
                gact = gact_pool.tile([P, ROWS], bf16, tag="gact")
                nc.scalar.activation(
                    gact, zg, AF.Gelu, bias=b1_sb[:, 16 + ut:17 + ut], scale=1.0
                )
                nc.vector.scalar_tensor_tensor(
                    ugT[:, ut, :], zu, b1_sb[:, ut:ut + 1], gact,
                    op0=OP.add, op1=OP.mult,
                )
                for it in range(4):
                    nc.tensor.matmul(
                        y_ps[:, it, :], ugT[:, ut, it * P:(it + 1) * P],
                        w2_sb[:, ut, :],
                        start=(ut == 0), stop=False,
                    )
            for it in range(4):
                nc.tensor.matmul(
                    y_ps[:, it, :], ones_row[0:1, 0:P], b2_row,
                    start=False, stop=True,
                )
                nc.vector.tensor_tensor(
                    out_sb[:, it, :], y_ps[:, it, :], x3[:, it, :], op=OP.add
                )

        for it_ in range(4):
            nc.sync.dma_start(out[:][it_ * P:(it_ + 1) * P, :], out_sb[:, it_, :])

    nc.compile()
    return nc


def _prep_shared(inputs):
    """Pre-cast weights to bf16 and pre-arrange into SBUF layouts (host-side
    layout prep, shared by all cores)."""
    bf = ml_dtypes.bfloat16

    def pkn(w, ktiles):
        # [ktiles*128, n] f32 -> [128, ktiles, n] bf16
        n = w.shape[1]
        return np.ascontiguousarray(
            w.reshape(ktiles, P, n).transpose(1, 0, 2).astype(bf))

    shared = {}
    for l in (1, 2, 4):
        shared[f"nw{l}"] = pkn(np.asarray(inputs[f"n{l}_w"], np.float32), 4)
        shared[f"nb{l}"] = np.ascontiguousarray(inputs[f"n{l}_b"], np.float32)
    for a in (1, 2):
        ws = [pkn(np.asarray(inputs[f"a{a}_{w}"], np.float32), 4)
              for w in "qkvo"]
        shared[f"a{a}s"] = np.ascontiguousarray(np.stack(ws, axis=2))
        shared[f"a{a}ob"] = np.asarray(inputs[f"a{a}_ob"], np.float32).astype(bf)
    shared["ffw1"] = pkn(np.asarray(inputs["ff_w1"], np.float32), 4)
    shared["ffw2"] = pkn(np.asarray(inputs["ff_w2"], np.float32), 16)
    shared["ffb1"] = np.ascontiguousarray(
        np.asarray(inputs["ff_b1"], np.float32).reshape(32, P).T)
    shared["ffb2"] = np.asarray(inputs["ff_b2"], np.float32).astype(bf)
    return shared


def _shard_inputs(inputs):
    """Build the 8 per-core input maps."""
    bf = ml_dtypes.bfloat16
    x = np.ascontiguousarray(inputs["x"], dtype=np.float32)
    t = np.ascontiguousarray(inputs["t"], dtype=np.float32)
    cond = np.ascontiguousarray(inputs["cond"], dtype=np.float32)
    shared = _prep_shared(inputs)
    per_batch = {}
    for b in range(B):
        condT = cond[b].T.reshape(4, P, NCTX).transpose(1, 0, 2)
        tT = t[b, 0].reshape(4, P).T
        per_batch[b] = (
            np.ascontiguousarray(condT.astype(bf)),
            np.ascontiguousarray(tT.astype(bf)),
        )
    in_maps = []
    for c in range(NCORES):
        b = c // 4
        r0 = (c % 4) * ROWS
        m = dict(shared)
        m["xb"] = np.ascontiguousarray(np.roll(x[b], -r0, axis=0))
        m["condT"], m["tT"] = per_batch[b]
        in_maps.append(m)
    return in_maps


def kernel(**inputs) -> np.ndarray:
    if "nc" not in _CACHED:
        _CACHED["nc"] = build()
    nc = _CACHED["nc"]
    in_maps = _shard_inputs(inputs)
    res = run_bass_kernel_spmd(nc, in_maps, core_ids=list(range(NCORES)))
    outs = [res.results[c]["out"] for c in range(NCORES)]
    full = np.concatenate(outs, axis=0).reshape(B, N, D)
    return full.astype(np.float32)


# revision 25
# speedup vs baseline: 1.2599x; 1.0568x over previous
"""BasicTransformerBlock on 8 TRN2 NeuronCores.

Sharding: sequence-parallel, zero collectives. The [B=2, N=2048, D=512]
residual stream is split into 8 row-blocks of 512 (4 cores per batch
element). Every core recomputes the cheap batch-wide work it needs
(adaln1 + K/V projections over its batch's 2048 rows, cond K/V), and does
attention / FFN only for its own 512 query rows.

Host-side prep (part of kernel()'s sharding layer, shared across cores):
weights are pre-cast to bf16 and pre-rearranged into the [partition, ktile,
cols] SBUF layouts, and cond is pre-transposed, so every DMA is a fast
contiguous HWDGE transfer (no software-DGE casting scatter loads) and no
on-chip cond transposes are needed. x stays f32 (layernorm stats need it).
Per-core x is pre-rotated with np.roll so "own" rows are always rows 0:512;
attention is permutation-invariant over keys, so rolled K/V is fine.

Attention: transposed scores sT[j, i] so exp() runs on ScalarE straight out
of PSUM. Score matmuls are DENSE (K=128): both heads of a pair are stacked
in the stationary operand and the query operand is zero-padded per head.
The PE_HAM activity monitor only grants the 2.4 GHz clock to full-array
matmuls; K=64 attention matmuls would run the whole phase at 1.2 GHz.
The 65th v-column of ones makes the softmax denominator fall out of the
attn@v matmul.

PSUM during attention: a 6-slot score ring (6 banks) + 2 accumulator
banks. exp is issued per 3-slot granule ([128, 3, 512] per ACTIVATE) to
amortize ACT instruction overhead, and the loop is skewed exp(g-1) /
scores(g) / av(g-2) so the PE batch for step g only depends on ACT work
from step g-2 -- the engines overlap instead of ping-ponging.
"""

import contextlib

import ml_dtypes
import numpy as np

import concourse.bass as bass
import concourse.mybir as mybir
import concourse.tile as tile
from concourse import bacc
from concourse.bass_utils import run_bass_kernel_spmd
from concourse.masks import make_identity

dt = mybir.dt
AF = mybir.ActivationFunctionType
OP = mybir.AluOpType

B, N, D = 2, 2048, 512
NCTX = 1024          # cond length
H = 8                # heads
HD = D // H          # 64
EPS = 1e-5
P = 128              # partitions
NCORES = 8
ROWS = 512           # own rows per core
NB = N               # batch rows per core (2048)
SCALE = HD ** -0.5   # 0.125

f32 = dt.float32
bf16 = dt.bfloat16

_CACHED = {}


def _adaln_stats(nc, stat_pool, src_tiles, n_tiles, eps_sb, chunk=4):
    """bn_stats/aggr + rstd/nmr for n_tiles row-tiles. Returns (rstd_all, nmr_all)."""
    mv_all = stat_pool.tile([P, n_tiles, 2], f32)
    rstd_all = stat_pool.tile([P, n_tiles], f32)
    nmr_all = stat_pool.tile([P, n_tiles], f32)
    for c0 in range(0, n_tiles, chunk):
        for it in range(c0, c0 + chunk):
            stats = stat_pool.tile([P, 6], f32, tag="stats")
            nc.vector.bn_stats(stats, src_tiles(it))
            nc.vector.bn_aggr(mv_all[:, it, :], stats)
        cs = slice(c0, c0 + chunk)
        nc.scalar.activation(rstd_all[:, cs], mv_all[:, cs, 1], AF.Sqrt,
                             bias=eps_sb, scale=1.0)
        nc.vector.reciprocal(rstd_all[:, cs], rstd_all[:, cs])
        nc.vector.scalar_tensor_tensor(
            nmr_all[:, cs], mv_all[:, cs, 0], -1.0, rstd_all[:, cs],
            op0=OP.mult, op1=OP.mult,
        )
    return rstd_all, nmr_all


def _adaln_apply_tile(nc, xn_pool, pst_pool, src, it, ab, rstd_all, nmr_all,
                      hT, ident_bf16, on_act=False):
    """One tile: xn = (x-mean)*rstd -> PE transpose -> (1+scale)/shift -> hT."""
    xn = xn_pool.tile([P, 512], bf16, tag="xn")
    if on_act:
        nc.scalar.activation(xn, src, AF.Identity,
                             bias=nmr_all[:, it:it + 1],
                             scale=rstd_all[:, it:it + 1])
    else:
        nc.vector.tensor_scalar(
            xn, src, rstd_all[:, it:it + 1], nmr_all[:, it:it + 1],
            op0=OP.mult, op1=OP.add,
        )
    xnt = pst_pool.tile([P, 4, P], bf16, tag="xnt")
    for b in range(4):
        nc.tensor.transpose(xnt[:, b, :], xn[:, b * P:(b + 1) * P], ident_bf16)
    for b in range(4):
        nc.vector.tensor_scalar(
            hT[:, b, it * P:(it + 1) * P], xnt[:, b, :],
            ab[:, b:b + 1], ab[:, 4 + b:5 + b],
            op0=OP.mult, op1=OP.add,
        )


def _adaln_to_hT(nc, tc, src_tiles, n_tiles, ab, hT, ident_bf16, eps_sb, name):
    with contextlib.ExitStack() as actx:
        stat_pool = actx.enter_context(tc.tile_pool(name=f"{name}_stat", bufs=4))
        xn_pool = actx.enter_context(tc.tile_pool(name=f"{name}_xn", bufs=3))
        pst_pool = actx.enter_context(
            tc.tile_pool(name=f"{name}_pst", bufs=2, space="PSUM"))
        rstd, nmr = _adaln_stats(nc, stat_pool, src_tiles, n_tiles, eps_sb,
                                 chunk=n_tiles)
        for it in range(n_tiles):
            _adaln_apply_tile(nc, xn_pool, pst_pool, src_tiles(it), it, ab,
                              rstd, nmr, hT, ident_bf16, on_act=False)


def _attention(nc, tc, act, qpad, kT, v, njt, wo, ob_row, ones_row,
               x_res, x_out, name):
    """Dense-score attention for 8 heads (4 pairs) over own 512 rows.

    qpad: [128, 4, 2, ROWS] bf16 zero-padded per head half.
    kT:   [128, 4, njt*128] bf16 (partitions = paired head dims).
    v:    [128, njt, 8, 65] bf16 (col 64 of each head = 1.0).
    Writes x_out = attn_out @ wo + ob + x_res  (all [128, 4, 512] f32).
    """
    av_all = act.tile([P, 4, ROWS], bf16, tag="tH")
    S = 2 * njt                       # score slots per pair
    G = (S + 2) // 3                  # exp granules per pair
    with (
        tc.tile_pool(name=f"{name}_ps_s", bufs=1, space="PSUM") as ps_s,
        tc.tile_pool(name=f"{name}_ps_av", bufs=1, space="PSUM") as ps_av,
        tc.tile_pool(name=f"{name}_et", bufs=3) as et_pool,
        tc.tile_pool(name=f"{name}_dn", bufs=4) as dn_pool,
        tc.tile_pool(name=f"{name}_rb", bufs=2) as rb_pool,
    ):
        sps = ps_s.tile([P, 6, ROWS], f32)
        rb_slot = 4

        def pair_tail(pht, pavp):
            # softmax denominators: row 64 of each accumulator. Broadcast
            # across partitions with K=1 matmuls into a retired ring slot,
            # then one fast reciprocal for the pair. Emitted AFTER the next
            # pair's first score granule so the PE never drains at pair
            # boundaries (a >3.4us PE gap re-throttles the HAM clock).
            for hl in range(2):
                dnm = dn_pool.tile([1, ROWS], bf16, tag="dnm")
                nc.vector.tensor_copy(dnm, pavp[HD:HD + 1, hl, :])
                nc.tensor.matmul(
                    sps[hl * HD:(hl + 1) * HD, rb_slot, :],
                    ones_row[0:1, 0:HD], dnm,
                    start=True, stop=True,
                )
            rb_sb = rb_pool.tile([P, ROWS], f32, tag="rb")
            nc.vector.reciprocal_approx_fast(rb_sb, sps[:, rb_slot, :])
            for hl in range(2):
                po = hl * HD
                nc.vector.scalar_tensor_tensor(
                    av_all[po:po + HD, pht, :],
                    pavp[0:HD, hl, :], 1.0, rb_sb[po:po + HD, :],
                    op0=OP.mult, op1=OP.mult,
                )

        prev = None
        for ht in range(4):           # head pair (2ht, 2ht+1)
            avp = ps_av.tile([HD + 1, 2, ROWS], f32, tag="av")
            ets = {}
            for g in range(G + 2):
                if 1 <= g <= G:
                    lo, hi = 3 * (g - 1), min(3 * (g - 1) + 3, S)
                    ng = hi - lo
                    et = et_pool.tile([P, 3, ROWS], bf16, tag="et")
                    ets[g - 1] = (et, lo, hi)
                    lo6 = lo % 6
                    nc.scalar.activation(
                        et[:, 0:ng, :], sps[:, lo6:lo6 + ng, :], AF.Exp,
                        scale=SCALE,
                    )
                if g == 1 and prev is not None:
                    # tail of the previous pair: its slot-4 write is then
                    # overwritten by this pair's g=1 scores (read-before-
                    # write deps keep it correct), so the PE stream never
                    # drains between pairs.
                    pair_tail(*prev)
                    prev = None
                if g < G:
                    lo, hi = 3 * g, min(3 * g + 3, S)
                    for s in range(lo, hi):
                        jt, hl = s // 2, s % 2
                        nc.tensor.matmul(
                            sps[:, s % 6, :],
                            kT[:, ht, jt * P:(jt + 1) * P],
                            qpad[:, ht, hl, :],
                            start=True, stop=True,
                        )
                if g >= 2:
                    et, lo, hi = ets.pop(g - 2)
                    for i, s in enumerate(range(lo, hi)):
                        jt, hl = s // 2, s % 2
                        nc.tensor.matmul(
                            avp[:, hl, :], v[:, jt, 2 * ht + hl, :],
                            et[:, i, :],
                            start=(jt == 0), stop=(jt == njt - 1),
                        )
            prev = (ht, avp)
        pair_tail(*prev)
    # out-projection + bias + residual
    with tc.tile_pool(name=f"{name}_ps_o", bufs=2, space="PSUM") as ps_o:
        for it in range(4):
            ps = ps_o.tile([P, D], f32, tag="o")
            for dt_ in range(4):
                nc.tensor.matmul(
                    ps, av_all[:, dt_, it * P:(it + 1) * P], wo[:, dt_, :],
                    start=(dt_ == 0), stop=False,
                )
            nc.tensor.matmul(
                ps, ones_row[0:1, 0:P], ob_row, start=False, stop=True,
            )
            nc.vector.tensor_tensor(x_out[:, it, :], ps, x_res[:, it, :], op=OP.add)


def build():
    nc = bacc.Bacc(None, target_bir_lowering=False)

    # -------- I/O (weights arrive pre-cast/pre-arranged from the host) ----
    xb = nc.dram_tensor("xb", [NB, D], f32, kind="ExternalInput")
    condT_d = nc.dram_tensor("condT", [P, 4, NCTX], bf16, kind="ExternalInput")
    tT_d = nc.dram_tensor("tT", [P, 4], bf16, kind="ExternalInput")
    nw_d = {}
    nb_d = {}
    for l in (1, 2, 4):
        nw_d[l] = nc.dram_tensor(f"nw{l}", [P, 4, 2 * D], bf16,
                                 kind="ExternalInput")
        nb_d[l] = nc.dram_tensor(f"nb{l}", [2 * D], f32, kind="ExternalInput")
    as_d = {}
    aob_d = {}
    for a in (1, 2):
        as_d[a] = nc.dram_tensor(f"a{a}s", [P, 4, 4, D], bf16,
                                 kind="ExternalInput")
        aob_d[a] = nc.dram_tensor(f"a{a}ob", [D], bf16, kind="ExternalInput")
    ffw1_d = nc.dram_tensor("ffw1", [P, 4, 8 * D], bf16, kind="ExternalInput")
    ffw2_d = nc.dram_tensor("ffw2", [P, 16, D], bf16, kind="ExternalInput")
    ffb1_d = nc.dram_tensor("ffb1", [P, 32], f32, kind="ExternalInput")
    ffb2_d = nc.dram_tensor("ffb2", [D], bf16, kind="ExternalInput")
    out = nc.dram_tensor("out", [ROWS, D], f32, kind="ExternalOutput")

    with tile.TileContext(nc) as tc, contextlib.ExitStack() as ctx:
        const = ctx.enter_context(tc.tile_pool(name="const", bufs=1))
        wpool = ctx.enter_context(tc.tile_pool(name="wpool", bufs=1))
        act = ctx.enter_context(tc.tile_pool(name="act", bufs=1))
        xr_pool = ctx.enter_context(tc.tile_pool(name="xrp", bufs=6))
        n1_stat = ctx.enter_context(tc.tile_pool(name="n1_stat", bufs=4))

        ident_bf16 = const.tile([P, P], bf16)
        make_identity(nc, ident_bf16)
        ident_f32 = const.tile([P, P], f32)
        make_identity(nc, ident_f32)
        ones_row = const.tile([1, P], bf16)
        nc.vector.memset(ones_row, 1.0)
        eps_sb = const.tile([P, 1], f32)
        nc.vector.memset(eps_sb, EPS)

        # ---------------- DMA issue order --------------------------------
        # qACT: weights in first-use order.  qSP: x, small rows, ff tail.
        tT = const.tile([P, 4], bf16)
        nc.scalar.dma_start(tT, tT_d[:])
        ab = {}
        with (
            tc.tile_pool(name="nwp", bufs=1) as nwp,
            tc.tile_pool(name="embp", bufs=1) as embp,
        ):
            nw_sb = {}
            for l in (1, 2):
                nw_sb[l] = nwp.tile([P, 4, 2 * D], bf16, tag=f"nw{l}",
                                    name=f"nw_sb{l}")
                nc.scalar.dma_start(nw_sb[l], nw_d[l][:])

            a_sb = {}
            stacks = {}
            for a, wtag in ((1, "wbig1"), (2, "wbig2")):
                stack = wpool.tile([P, 4, 4, D], bf16, tag=wtag,
                                   name=f"a{a}stk")
                stacks[a] = stack
                for wi, w in enumerate("qkvo"):
                    a_sb[a, w] = stack[:, :, wi, :]
            pass
            for a in (1, 2):
                ob = wpool.tile([1, D], bf16, tag=f"a{a}ob", name=f"a{a}ob_sb")
                a_sb[a, "ob"] = ob

            h1T = act.tile([P, 4, NB], bf16, tag="tA")
            own_x = act.tile([P, 4, D], f32, tag="tE")
            x_tiles = {}
            for it in range(16):
                if it < 4:
                    dst = own_x[:, it, :]
                else:
                    dst = xr_pool.tile([P, D], f32, tag="xr", name=f"xr{it}")
                nc.gpsimd.dma_start(dst, xb[:][it * P:(it + 1) * P, :])
                x_tiles[it] = dst

            # condT + k2T share the tX region
            ctk2 = act.tile([P, 2, 4, NCTX], bf16, tag="tX")
            condT = ctk2[:, 0, :, :]
            k2T = ctk2[:, 1, :, :]
            # big weights + condT on the otherwise-idle SWDGE queue: HWDGE
            # trigger instructions cost ~2.5us EACH on their engine's queue
            # and were starving the ACT stats chain.
            nc.gpsimd.dma_start(stacks[1], as_d[1][:])
            nc.gpsimd.dma_start(a_sb[1, "ob"],
                                aob_d[1][:].rearrange("(a n) -> a n", a=1))
            nc.gpsimd.dma_start(stacks[2], as_d[2][:])
            nc.gpsimd.dma_start(a_sb[2, "ob"],
                                aob_d[2][:].rearrange("(a n) -> a n", a=1))
            nc.gpsimd.dma_start(condT, condT_d[:])
            nw_sb[4] = nwp.tile([P, 4, 2 * D], bf16, tag="nw1", name="nw_sb4")
            nc.gpsimd.dma_start(nw_sb[4], nw_d[4][:])

            nb_row = {}
            for l in (1, 2, 4):
                nb_row[l] = embp.tile([1, 2 * D], f32, tag="nbrow",
                                      name=f"nb_row{l}")
                nc.sync.dma_start(nb_row[l],
                                  nb_d[l][:].rearrange("(a n) -> a n", a=1))
            b1_sb = const.tile([P, 32], f32)
            nc.sync.dma_start(b1_sb, ffb1_d[:])
            b2_row = const.tile([1, D], bf16)
            nc.sync.dma_start(b2_row, ffb2_d[:].rearrange("(a n) -> a n", a=1))
            # ff weights on qSP after x: landed long before the FFN needs
            # them, but the buffers alias a1s/a2s so they wait on attention.
            w1_sb = wpool.tile([P, 4, 8 * D], bf16, tag="wbig1")
            nc.sync.dma_start(w1_sb, ffw1_d[:])
            w2_sb = wpool.tile([P, 16, D], bf16, tag="wbig2")
            nc.sync.dma_start(w2_sb, ffw2_d[:])

            # PE warmup: dependency-free matmuls fill the startup DMA window
            with tc.tile_pool(name="warm", bufs=1, space="PSUM") as warm_pool:
                wps = warm_pool.tile([P, P], f32)
                for _ in range(30):
                    nc.tensor.matmul(wps, ident_bf16, ident_bf16,
                                     start=True, stop=True)

            # adaln1 stats first: independent of norm weights
            rstd1, nmr1 = _adaln_stats(nc, n1_stat, lambda it: x_tiles[it],
                                       16, eps_sb)

            # ------------- norm scale/shift params -----------------------
            with tc.tile_pool(name="ps_emb", bufs=2, space="PSUM") as ps_emb:
                for l in (1, 2, 4):
                    emb_ps = ps_emb.tile([1, 2 * D], f32, tag="embps")
                    for half in range(2):
                        for kt in range(4):
                            nc.tensor.matmul(
                                emb_ps[:, half * D:(half + 1) * D],
                                tT[:, kt:kt + 1],
                                nw_sb[l][:, kt, half * D:(half + 1) * D],
                                start=(kt == 0), stop=(kt == 3),
                            )
                    emb_row = embp.tile([1, 2 * D], f32, tag="embrow",
                                        name=f"emb_row{l}")
                    nc.vector.tensor_tensor(emb_row, emb_ps, nb_row[l],
                                            op=OP.add)
                    ab_l = const.tile([P, 8], f32, tag=f"ab{l}")
                    for col in range(8):
                        tp = ps_emb.tile([P, 1], f32, tag="embT")
                        nc.tensor.transpose(
                            tp, emb_row[0:1, col * P:(col + 1) * P],
                            ident_f32[0:1, 0:1]
                        )
                        nc.vector.tensor_scalar(
                            ab_l[:, col:col + 1], tp,
                            1.0 if col < 4 else 0.0, None, op0=OP.add,
                        )
                    ab[l] = ab_l

        # ---------------- adaln1 apply + projections, interleaved --------
        k1T = act.tile([P, 4, NB], bf16, tag="tB")
        v1 = act.tile([P, 16, H, HD + 1], bf16, tag="tC")
        q1pad = act.tile([P, 4, 2, ROWS], bf16, tag="tD")
        nc.gpsimd.memset(v1[:, :, :, HD:HD + 1], 1.0)
        nc.gpsimd.memset(q1pad, 0.0)
        with (
            tc.tile_pool(name="n1_xn", bufs=3) as xn_pool,
            tc.tile_pool(name="n1_pst", bufs=2, space="PSUM") as pst_pool,
            tc.tile_pool(name="ps_proj1", bufs=4, space="PSUM") as ps_proj,
        ):
            for jc in range(4):
                for it in range(4 * jc, 4 * jc + 4):
                    _adaln_apply_tile(nc, xn_pool, pst_pool, x_tiles[it], it,
                                      ab[1], rstd1, nmr1, h1T, ident_bf16,
                                      on_act=True)
                for dt_ in range(4):
                    ps = ps_proj.tile([P, 512], f32, tag="proj")
                    for kt in range(4):
                        nc.tensor.matmul(
                            ps,
                            a_sb[1, "k"][:, kt, dt_ * P:(dt_ + 1) * P],
                            h1T[:, kt, jc * 512:(jc + 1) * 512],
                            start=(kt == 0), stop=(kt == 3),
                        )
                    nc.vector.tensor_copy(
                        k1T[:, dt_, jc * 512:(jc + 1) * 512], ps
                    )
                for jt in range(4 * jc, 4 * jc + 4):
                    ps = ps_proj.tile([P, 512], f32, tag="proj")
                    for kt in range(4):
                        nc.tensor.matmul(
                            ps,
                            h1T[:, kt, jt * P:(jt + 1) * P],
                            a_sb[1, "v"][:, kt, :],
                            start=(kt == 0), stop=(kt == 3),
                        )
                    nc.vector.tensor_copy(
                        v1[:, jt, :, 0:HD], ps.rearrange("p (h d) -> p h d", h=H)
                    )
                if jc == 0:
                    for dt_ in range(4):
                        ps = ps_proj.tile([P, 512], f32, tag="proj")
                        for kt in range(4):
                            nc.tensor.matmul(
                                ps,
                                a_sb[1, "q"][:, kt, dt_ * P:(dt_ + 1) * P],
                                h1T[:, kt, 0:ROWS],
                                start=(kt == 0), stop=(kt == 3),
                            )
                        nc.vector.tensor_copy(q1pad[0:HD, dt_, 0, :], ps[0:HD, :])
                        nc.vector.tensor_copy(q1pad[HD:P, dt_, 1, :], ps[HD:P, :])

        # ---------------- attention 1 ------------------------------------
        x2 = act.tile([P, 4, D], f32, tag="tF")
        _attention(nc, tc, act, q1pad, k1T, v1, 16, a_sb[1, "o"],
                   a_sb[1, "ob"], ones_row, own_x, x2, "att1")

        # ------- cross-attn prep: k2T, v2 (independent of x) -------------
        v2 = act.tile([P, 8, H, HD + 1], bf16, tag="tI")
        nc.gpsimd.memset(v2[:, :, :, HD:HD + 1], 1.0)
        with tc.tile_pool(name="ps_proj2a", bufs=4, space="PSUM") as ps_proj:
            for dt_ in range(4):
                for jc in range(2):
                    ps = ps_proj.tile([P, 512], f32, tag="proj")
                    for kt in range(4):
                        nc.tensor.matmul(
                            ps,
                            a_sb[2, "k"][:, kt, dt_ * P:(dt_ + 1) * P],
                            condT[:, kt, jc * 512:(jc + 1) * 512],
                            start=(kt == 0), stop=(kt == 3),
                        )
                    nc.vector.tensor_copy(
                        k2T[:, dt_, jc * 512:(jc + 1) * 512], ps
                    )
            for jt in range(8):
                ps = ps_proj.tile([P, 512], f32, tag="proj")
                for kt in range(4):
                    nc.tensor.matmul(
                        ps,
                        condT[:, kt, jt * P:(jt + 1) * P],
                        a_sb[2, "v"][:, kt, :],
                        start=(kt == 0), stop=(kt == 3),
                    )
                nc.vector.tensor_copy(
                    v2[:, jt, :, 0:HD], ps.rearrange("p (h d) -> p h d", h=H)
                )

        # ---------------- adaln2 + cross-attn ----------------------------
        h2T = act.tile([P, 4, ROWS], bf16, tag="tH")
        _adaln_to_hT(nc, tc, lambda it: x2[:, it, :], 4, ab[2], h2T,
                     ident_bf16, eps_sb, "n2")

        q2pad = act.tile([P, 4, 2, ROWS], bf16, tag="tD")
        nc.gpsimd.memset(q2pad, 0.0)
        with tc.tile_pool(name="ps_proj2b", bufs=2, space="PSUM") as ps_proj:
            for dt_ in range(4):
                ps = ps_proj.tile([P, 512], f32, tag="proj")
                for kt in range(4):
                    nc.tensor.matmul(
                        ps,
                        a_sb[2, "q"][:, kt, dt_ * P:(dt_ + 1) * P],
                        h2T[:, kt, :],
                        start=(kt == 0), stop=(kt == 3),
                    )
                nc.vector.tensor_copy(q2pad[0:HD, dt_, 0, :], ps[0:HD, :])
                nc.vector.tensor_copy(q2pad[HD:P, dt_, 1, :], ps[HD:P, :])

        x3 = act.tile([P, 4, D], f32, tag="tG")
        _attention(nc, tc, act, q2pad, k2T, v2, 8, a_sb[2, "o"],
                   a_sb[2, "ob"], ones_row, x2, x3, "att2")

        # ---------------- adaln3 + GEGLU FFN -----------------------------
        h3T = act.tile([P, 4, ROWS], bf16, tag="tJ")
        _adaln_to_hT(nc, tc, lambda it: x3[:, it, :], 4, ab[4], h3T,
                     ident_bf16, eps_sb, "n4")

        # per-ut pipeline: zu/zg -> gelu/stt -> 4 y-accumulator matmuls.
        # y accumulates in 4 persistent PSUM banks across all 16 ut chunks.
        ugT = act.tile([P, 16, ROWS], bf16, tag="tA")
        out_sb = act.tile([P, 4, D], f32, tag="tC")
        with (
            tc.tile_pool(name="ps_z", bufs=4, space="PSUM") as ps_z,
            tc.tile_pool(name="ps_y", bufs=1, space="PSUM") as ps_y,
            tc.tile_pool(name="gact", bufs=3) as gact_pool,
        ):
            y_ps = ps_y.tile([P, 4, D], f32)
            for ut in range(16):
                zu = ps_z.tile([P, ROWS], f32, tag="z")
                zg = ps_z.tile([P, ROWS], f32, tag="z")
                for kt in range(4):
                    nc.tensor.matmul(
                        zu, w1_sb[:, kt, ut * P:(ut + 1) * P],
                        h3T[:, kt, :], start=(kt == 0), stop=(kt == 3),
                    )
                for kt in range(4):
                    nc.tensor.matmul(
                        zg, w1_sb[:, kt, (16 + ut) * P:(17 + ut) * P],
                        h3T[:, kt, :], start=(kt == 0), stop=(kt == 3),
                    )
                gact = gact_pool.tile([P, ROWS], bf16, tag="gact")
                nc.scalar.activation(
                    gact, zg, AF.Gelu, bias=b1_sb[:, 16 + ut:17 + ut], scale=1.0
                )
                nc.vector.scalar_tensor_tensor(
                    ugT[:, ut, :], zu, b1_sb[:, ut:ut + 1], gact,
                    op0=OP.add, op1=OP.mult,
                )
                for it in range(4):
                    nc.tensor.matmul(
                        y_ps[:, it, :], ugT[:, ut, it * P:(it + 1) * P],
                        w2_sb[:, ut, :],
                        start=(ut == 0), stop=False,
                    )
            for it in range(4):
                nc.tensor.matmul(
                    y_ps[:, it, :], ones_row[0:1, 0:P], b2_row,
                    start=False, stop=True,
                )
                nc.vector.tensor_tensor(
                    out_sb[:, it, :], y_ps[:, it, :], x3[:, it, :], op=OP.add
                )

        for it_ in range(4):
            nc.sync.dma_start(out[:][it_ * P:(it_ + 1) * P, :], out_sb[:, it_, :])

    nc.compile()
    return nc


def _prep_shared(inputs):
    """Pre-cast weights to bf16 and pre-arrange into SBUF layouts (host-side
    layout prep, shared by all cores)."""
    bf = ml_dtypes.bfloat16

    def pkn(w, ktiles):
        # [ktiles*128, n] f32 -> [128, ktiles, n] bf16
        n = w.shape[1]
        return np.ascontiguousarray(
            w.reshape(ktiles, P, n).transpose(1, 0, 2).astype(bf))

    shared = {}
    for l in (1, 2, 4):
        shared[f"nw{l}"] = pkn(np.asarray(inputs[f"n{l}_w"], np.float32), 4)
        shared[f"nb{l}"] = np.ascontiguousarray(inputs[f"n{l}_b"], np.float32)
    for a in (1, 2):
        ws = [pkn(np.asarray(inputs[f"a{a}_{w}"], np.float32), 4)
              for w in "qkvo"]
        shared[f"a{a}s"] = np.ascontiguousarray(np.stack(ws, axis=2))
        shared[f"a{a}ob"] = np.asarray(inputs[f"a{a}_ob"], np.float32).astype(bf)
    shared["ffw1"] = pkn(np.asarray(inputs["ff_w1"], np.float32), 4)
    shared["ffw2"] = pkn(np.asarray(inputs["ff_w2"], np.float32), 16)
    shared["ffb1"] = np.ascontiguousarray(
        np.asarray(inputs["ff_b1"], np.float32).reshape(32, P).T)
    shared["ffb2"] = np.asarray(inputs["ff_b2"], np.float32).astype(bf)
    return shared


def _shard_inputs(inputs):
    """Build the 8 per-core input maps."""
    bf = ml_dtypes.bfloat16
    x = np.ascontiguousarray(inputs["x"], dtype=np.float32)
    t = np.ascontiguousarray(inputs["t"], dtype=np.float32)
    cond = np.ascontiguousarray(inputs["cond"], dtype=np.float32)
    shared = _prep_shared(inputs)
    per_batch = {}
    for b in range(B):
        condT = cond[b].T.reshape(4, P, NCTX).transpose(1, 0, 2)
        tT = t[b, 0].reshape(4, P).T
        per_batch[b] = (
            np.ascontiguousarray(condT.astype(bf)),
            np.ascontiguousarray(tT.astype(bf)),
        )
    in_maps = []
    for c in range(NCORES):
        b = c // 4
        r0 = (c % 4) * ROWS
        m = dict(shared)
        m["xb"] = np.ascontiguousarray(np.roll(x[b], -r0, axis=0))
        m["condT"], m["tT"] = per_batch[b]
        in_maps.append(m)
    return in_maps


def kernel(**inputs) -> np.ndarray:
    if "nc" not in _CACHED:
        _CACHED["nc"] = build()
    nc = _CACHED["nc"]
    in_maps = _shard_inputs(inputs)
    res = run_bass_kernel_spmd(nc, in_maps, core_ids=list(range(NCORES)))
    outs = [res.results[c]["out"] for c in range(NCORES)]
    full = np.concatenate(outs, axis=0).reshape(B, N, D)
    return full.astype(np.float32)


# revision 31
# speedup vs baseline: 1.4561x; 1.1557x over previous
"""BasicTransformerBlock on 8 TRN2 NeuronCores.

Sharding: sequence-parallel, zero collectives. The [B=2, N=2048, D=512]
residual stream is split into 8 row-blocks of 512 (4 cores per batch
element). Every core recomputes the cheap batch-wide work it needs
(adaln1 + K/V projections over its batch's 2048 rows, cond K/V), and does
attention / FFN only for its own 512 query rows.

Host-side prep (part of kernel()'s sharding layer, shared across cores):
weights are pre-cast to bf16 and pre-rearranged into the [partition, ktile,
cols] SBUF layouts, and cond is pre-transposed, so every DMA is a fast
contiguous HWDGE transfer (no software-DGE casting scatter loads) and no
on-chip cond transposes are needed. x stays f32 (layernorm stats need it).
Per-core x is pre-rotated with np.roll so "own" rows are always rows 0:512;
attention is permutation-invariant over keys, so rolled K/V is fine.

Attention: transposed scores sT[j, i] so exp() runs on ScalarE straight out
of PSUM. Score matmuls are DENSE (K=128): both heads of a pair are stacked
in the stationary operand and the query operand is zero-padded per head.
The PE_HAM activity monitor only grants the 2.4 GHz clock to full-array
matmuls; K=64 attention matmuls would run the whole phase at 1.2 GHz.
The 65th v-column of ones makes the softmax denominator fall out of the
attn@v matmul.

PSUM during attention: a 6-slot score ring (6 banks) + 2 accumulator
banks. exp is issued per 3-slot granule ([128, 3, 512] per ACTIVATE) to
amortize ACT instruction overhead, and the loop is skewed exp(g-1) /
scores(g) / av(g-2) so the PE batch for step g only depends on ACT work
from step g-2 -- the engines overlap instead of ping-ponging.
"""

import contextlib

import ml_dtypes
import numpy as np

import concourse.bass as bass
import concourse.mybir as mybir
import concourse.tile as tile
from concourse import bacc
from concourse.bass_utils import run_bass_kernel_spmd
from concourse.masks import make_identity

dt = mybir.dt
AF = mybir.ActivationFunctionType
OP = mybir.AluOpType

B, N, D = 2, 2048, 512
NCTX = 1024          # cond length
H = 8                # heads
HD = D // H          # 64
EPS = 1e-5
P = 128              # partitions
NCORES = 8
ROWS = 512           # own rows per core
NB = N               # batch rows per core (2048)
SCALE = HD ** -0.5   # 0.125

f32 = dt.float32
bf16 = dt.bfloat16

_CACHED = {}


def _adaln_stats(nc, stat_pool, src_tiles, n_tiles, eps_sb, chunk=4):
    """bn_stats/aggr + rstd/nmr for n_tiles row-tiles. Returns (rstd_all, nmr_all)."""
    mv_all = stat_pool.tile([P, n_tiles, 2], f32)
    rstd_all = stat_pool.tile([P, n_tiles], f32)
    nmr_all = stat_pool.tile([P, n_tiles], f32)
    for c0 in range(0, n_tiles, chunk):
        for it in range(c0, c0 + chunk):
            stats = stat_pool.tile([P, 6], f32, tag="stats")
            nc.vector.bn_stats(stats, src_tiles(it))
            nc.vector.bn_aggr(mv_all[:, it, :], stats)
        cs = slice(c0, c0 + chunk)
        nc.scalar.activation(rstd_all[:, cs], mv_all[:, cs, 1], AF.Sqrt,
                             bias=eps_sb, scale=1.0)
        nc.vector.reciprocal(rstd_all[:, cs], rstd_all[:, cs])
        nc.vector.scalar_tensor_tensor(
            nmr_all[:, cs], mv_all[:, cs, 0], -1.0, rstd_all[:, cs],
            op0=OP.mult, op1=OP.mult,
        )
    return rstd_all, nmr_all


def _adaln_apply_tile(nc, xn_pool, pst_pool, src, it, ab, rstd_all, nmr_all,
                      hT, ident_bf16, on_act=False):
    """One tile: xn = (x-mean)*rstd -> PE transpose -> (1+scale)/shift -> hT."""
    xn = xn_pool.tile([P, 512], bf16, tag="xn")
    if on_act:
        nc.scalar.activation(xn, src, AF.Identity,
                             bias=nmr_all[:, it:it + 1],
                             scale=rstd_all[:, it:it + 1])
    else:
        nc.vector.tensor_scalar(
            xn, src, rstd_all[:, it:it + 1], nmr_all[:, it:it + 1],
            op0=OP.mult, op1=OP.add,
        )
    xnt = pst_pool.tile([P, 4, P], bf16, tag="xnt")
    for b in range(4):
        nc.tensor.transpose(xnt[:, b, :], xn[:, b * P:(b + 1) * P], ident_bf16)
    for b in range(4):
        nc.vector.tensor_scalar(
            hT[:, b, it * P:(it + 1) * P], xnt[:, b, :],
            ab[:, b:b + 1], ab[:, 4 + b:5 + b],
            op0=OP.mult, op1=OP.add,
        )


def _adaln_to_hT(nc, tc, src_tiles, n_tiles, ab, hT, ident_bf16, eps_sb, name):
    with contextlib.ExitStack() as actx:
        stat_pool = actx.enter_context(tc.tile_pool(name=f"{name}_stat", bufs=4))
        xn_pool = actx.enter_context(tc.tile_pool(name=f"{name}_xn", bufs=3))
        pst_pool = actx.enter_context(
            tc.tile_pool(name=f"{name}_pst", bufs=2, space="PSUM"))
        rstd, nmr = _adaln_stats(nc, stat_pool, src_tiles, n_tiles, eps_sb,
                                 chunk=n_tiles)
        for it in range(n_tiles):
            _adaln_apply_tile(nc, xn_pool, pst_pool, src_tiles(it), it, ab,
                              rstd, nmr, hT, ident_bf16, on_act=False)


def _attention(nc, tc, act, qpad, kT, v, njt, wo, ob_row, ones_row,
               x_res, x_out, name):
    """Dense-score attention for 8 heads (4 pairs) over own 512 rows.

    qpad: [128, 4, 2, ROWS] bf16 zero-padded per head half.
    kT:   [128, 4, njt*128] bf16 (partitions = paired head dims).
    v:    [128, njt, 8, 65] bf16 (col 64 of each head = 1.0).
    Writes x_out = attn_out @ wo + ob + x_res  (all [128, 4, 512] f32).
    """
    av_all = act.tile([P, 4, ROWS], bf16, tag="tH")
    S = 2 * njt                       # score slots per pair
    G = (S + 2) // 3                  # exp granules per pair
    with (
        tc.tile_pool(name=f"{name}_ps_s", bufs=2, space="PSUM") as ps_s,
        tc.tile_pool(name=f"{name}_ps_av", bufs=1, space="PSUM") as ps_av,
        tc.tile_pool(name=f"{name}_et", bufs=3) as et_pool,
        tc.tile_pool(name=f"{name}_dn", bufs=4) as dn_pool,
        tc.tile_pool(name=f"{name}_rb", bufs=2) as rb_pool,
    ):
        def pair_tail(pht, pavp):
            # softmax denominators: row 64 of each accumulator. Broadcast
            # across partitions with K=1 matmuls into a score-pool tile
            # (keeps the ping-pong rotation in phase), then one fast
            # reciprocal for the pair. Emitted at the next pair's start so
            # the PE never drains at pair boundaries (a >3.4us PE gap
            # re-throttles the HAM clock).
            rbt = ps_s.tile([P, 3, ROWS], f32, tag="sgr", name="rbt")
            rb_ps = rbt[:, 0, :]
            for hl in range(2):
                dnm = dn_pool.tile([1, ROWS], bf16, tag="dnm")
                nc.vector.tensor_copy(dnm, pavp[HD:HD + 1, hl, :])
                nc.tensor.matmul(
                    rb_ps[hl * HD:(hl + 1) * HD, :],
                    ones_row[0:1, 0:HD], dnm,
                    start=True, stop=True,
                )
            rb_sb = rb_pool.tile([P, ROWS], f32, tag="rb")
            nc.vector.reciprocal_approx_fast(rb_sb, rb_ps)
            for hl in range(2):
                po = hl * HD
                nc.vector.scalar_tensor_tensor(
                    av_all[po:po + HD, pht, :],
                    pavp[0:HD, hl, :], 1.0, rb_sb[po:po + HD, :],
                    op0=OP.mult, op1=OP.mult,
                )

        prev = None
        for ht in range(4):           # head pair (2ht, 2ht+1)
            if prev is not None:
                pair_tail(*prev)
                prev = None
            avp = ps_av.tile([P, 2, ROWS], f32, tag="av")
            ets = {}
            sgr = {}
            for g in range(G + 2):
                if 1 <= g <= G:
                    lo, hi = 3 * (g - 1), min(3 * (g - 1) + 3, S)
                    ng = hi - lo
                    et = et_pool.tile([P, 3, ROWS], bf16, tag="et")
                    ets[g - 1] = (et, lo, hi)
                    nc.scalar.activation(
                        et[:, 0:ng, :], sgr[g - 1][:, 0:ng, :], AF.Exp,
                        scale=SCALE,
                    )
                if g < G:
                    lo, hi = 3 * g, min(3 * g + 3, S)
                    sg = ps_s.tile([P, 3, ROWS], f32, tag="sgr")
                    sgr[g] = sg
                    for s in range(lo, hi):
                        jt, hl = s // 2, s % 2
                        nc.tensor.matmul(
                            sg[:, s - lo, :],
                            kT[:, ht, jt * P:(jt + 1) * P],
                            qpad[:, ht, hl, :],
                            start=True, stop=True,
                        )
                if g >= 2:
                    et, lo, hi = ets.pop(g - 2)
                    sgr.pop(g - 2, None)
                    for i, s in enumerate(range(lo, hi)):
                        jt, hl = s // 2, s % 2
                        nc.tensor.matmul(
                            avp[0:HD + 1, hl, :], v[:, jt, 2 * ht + hl, :],
                            et[:, i, :],
                            start=(jt == 0), stop=(jt == njt - 1),
                        )
            prev = (ht, avp)
        pair_tail(*prev)
    # out-projection + bias + residual
    with tc.tile_pool(name=f"{name}_ps_o", bufs=2, space="PSUM") as ps_o:
        for it in range(4):
            ps = ps_o.tile([P, D], f32, tag="o")
            for dt_ in range(4):
                nc.tensor.matmul(
                    ps, av_all[:, dt_, it * P:(it + 1) * P], wo[:, dt_, :],
                    start=(dt_ == 0), stop=False,
                )
            nc.tensor.matmul(
                ps, ones_row[0:1, 0:P], ob_row, start=False, stop=True,
            )
            nc.vector.tensor_tensor(x_out[:, it, :], ps, x_res[:, it, :], op=OP.add)


def build():
    nc = bacc.Bacc(None, target_bir_lowering=False)

    # -------- I/O (weights arrive pre-cast/pre-arranged from the host) ----
    xb = nc.dram_tensor("xb", [NB, D], f32, kind="ExternalInput")
    condT_d = nc.dram_tensor("condT", [P, 4, NCTX], bf16, kind="ExternalInput")
    tT_d = nc.dram_tensor("tT", [P, 4], bf16, kind="ExternalInput")
    nw_d = {}
    nb_d = {}
    for l in (1, 2, 4):
        nw_d[l] = nc.dram_tensor(f"nw{l}", [P, 4, 2 * D], bf16,
                                 kind="ExternalInput")
        nb_d[l] = nc.dram_tensor(f"nb{l}", [2 * D], f32, kind="ExternalInput")
    as_d = {}
    aob_d = {}
    for a in (1, 2):
        as_d[a] = nc.dram_tensor(f"a{a}s", [P, 4, 4, D], bf16,
                                 kind="ExternalInput")
        aob_d[a] = nc.dram_tensor(f"a{a}ob", [D], bf16, kind="ExternalInput")
    ffw1_d = nc.dram_tensor("ffw1", [P, 4, 8 * D], bf16, kind="ExternalInput")
    ffw2_d = nc.dram_tensor("ffw2", [P, 16, D], bf16, kind="ExternalInput")
    ffb1_d = nc.dram_tensor("ffb1", [P, 32], f32, kind="ExternalInput")
    ffb2_d = nc.dram_tensor("ffb2", [D], bf16, kind="ExternalInput")
    out = nc.dram_tensor("out", [ROWS, D], f32, kind="ExternalOutput")

    with tile.TileContext(nc) as tc, contextlib.ExitStack() as ctx:
        const = ctx.enter_context(tc.tile_pool(name="const", bufs=1))
        wpool = ctx.enter_context(tc.tile_pool(name="wpool", bufs=1))
        act = ctx.enter_context(tc.tile_pool(name="act", bufs=1))
        xr_pool = ctx.enter_context(tc.tile_pool(name="xrp", bufs=6))
        n1_stat = ctx.enter_context(tc.tile_pool(name="n1_stat", bufs=4))

        ident_bf16 = const.tile([P, P], bf16)
        make_identity(nc, ident_bf16)
        ident_f32 = const.tile([P, P], f32)
        make_identity(nc, ident_f32)
        ones_row = const.tile([1, P], bf16)
        nc.vector.memset(ones_row, 1.0)
        eps_sb = const.tile([P, 1], f32)
        nc.vector.memset(eps_sb, EPS)

        # ---------------- DMA issue order --------------------------------
        # qACT: weights in first-use order.  qSP: x, small rows, ff tail.
        tT = const.tile([P, 4], bf16)
        nc.scalar.dma_start(tT, tT_d[:])
        ab = {}
        with (
            tc.tile_pool(name="nwp", bufs=1) as nwp,
            tc.tile_pool(name="embp", bufs=1) as embp,
        ):
            nw_sb = {}
            for l in (1, 2):
                nw_sb[l] = nwp.tile([P, 4, 2 * D], bf16, tag=f"nw{l}",
                                    name=f"nw_sb{l}")
                nc.scalar.dma_start(nw_sb[l], nw_d[l][:])

            a_sb = {}
            stacks = {}
            for a, wtag in ((1, "wbig1"), (2, "wbig2")):
                stack = wpool.tile([P, 4, 4, D], bf16, tag=wtag,
                                   name=f"a{a}stk")
                stacks[a] = stack
                for wi, w in enumerate("qkvo"):
                    a_sb[a, w] = stack[:, :, wi, :]
            pass
            for a in (1, 2):
                ob = wpool.tile([1, D], bf16, tag=f"a{a}ob", name=f"a{a}ob_sb")
                a_sb[a, "ob"] = ob

            h1T = act.tile([P, 4, NB], bf16, tag="tA")
            own_x = act.tile([P, 4, D], f32, tag="tE")
            x_tiles = {}
            for it in range(16):
                if it < 4:
                    dst = own_x[:, it, :]
                else:
                    dst = xr_pool.tile([P, D], f32, tag="xr", name=f"xr{it}")
                nc.gpsimd.dma_start(dst, xb[:][it * P:(it + 1) * P, :])
                x_tiles[it] = dst

            # condT + k2T share the tX region
            ctk2 = act.tile([P, 2, 4, NCTX], bf16, tag="tX")
            condT = ctk2[:, 0, :, :]
            k2T = ctk2[:, 1, :, :]
            # big weights + condT on the otherwise-idle SWDGE queue: HWDGE
            # trigger instructions cost ~2.5us EACH on their engine's queue
            # and were starving the ACT stats chain.
            nc.gpsimd.dma_start(stacks[1], as_d[1][:])
            nc.gpsimd.dma_start(a_sb[1, "ob"],
                                aob_d[1][:].rearrange("(a n) -> a n", a=1))
            nc.gpsimd.dma_start(stacks[2], as_d[2][:])
            nc.gpsimd.dma_start(a_sb[2, "ob"],
                                aob_d[2][:].rearrange("(a n) -> a n", a=1))
            nc.gpsimd.dma_start(condT, condT_d[:])
            nw_sb[4] = nwp.tile([P, 4, 2 * D], bf16, tag="nw1", name="nw_sb4")
            nc.gpsimd.dma_start(nw_sb[4], nw_d[4][:])

            nb_row = {}
            for l in (1, 2, 4):
                nb_row[l] = embp.tile([1, 2 * D], f32, tag="nbrow",
                                      name=f"nb_row{l}")
                nc.sync.dma_start(nb_row[l],
                                  nb_d[l][:].rearrange("(a n) -> a n", a=1))
            b1_sb = const.tile([P, 32], f32)
            nc.sync.dma_start(b1_sb, ffb1_d[:])
            b2_row = const.tile([1, D], bf16)
            nc.sync.dma_start(b2_row, ffb2_d[:].rearrange("(a n) -> a n", a=1))
            # ff weights on qSP after x: landed long before the FFN needs
            # them, but the buffers alias a1s/a2s so they wait on attention.
            w1_sb = wpool.tile([P, 4, 8 * D], bf16, tag="wbig1")
            nc.sync.dma_start(w1_sb, ffw1_d[:])
            w2_sb = wpool.tile([P, 16, D], bf16, tag="wbig2")
            nc.sync.dma_start(w2_sb, ffw2_d[:])

            # PE warmup: dependency-free matmuls fill the startup DMA window
            with tc.tile_pool(name="warm", bufs=1, space="PSUM") as warm_pool:
                wps = warm_pool.tile([P, P], f32)
                for _ in range(30):
                    nc.tensor.matmul(wps, ident_bf16, ident_bf16,
                                     start=True, stop=True)

            # adaln1 stats first: independent of norm weights
            rstd1, nmr1 = _adaln_stats(nc, n1_stat, lambda it: x_tiles[it],
                                       16, eps_sb)

            # ------------- norm scale/shift params -----------------------
            with tc.tile_pool(name="ps_emb", bufs=2, space="PSUM") as ps_emb:
                for l in (1, 2, 4):
                    emb_ps = ps_emb.tile([1, 2 * D], f32, tag="embps")
                    for half in range(2):
                        for kt in range(4):
                            nc.tensor.matmul(
                                emb_ps[:, half * D:(half + 1) * D],
                                tT[:, kt:kt + 1],
                                nw_sb[l][:, kt, half * D:(half + 1) * D],
                                start=(kt == 0), stop=(kt == 3),
                            )
                    emb_row = embp.tile([1, 2 * D], f32, tag="embrow",
                                        name=f"emb_row{l}")
                    nc.vector.tensor_tensor(emb_row, emb_ps, nb_row[l],
                                            op=OP.add)
                    ab_l = const.tile([P, 8], f32, tag=f"ab{l}")
                    for col in range(8):
                        tp = ps_emb.tile([P, 1], f32, tag="embT")
                        nc.tensor.transpose(
                            tp, emb_row[0:1, col * P:(col + 1) * P],
                            ident_f32[0:1, 0:1]
                        )
                        nc.vector.tensor_scalar(
                            ab_l[:, col:col + 1], tp,
                            1.0 if col < 4 else 0.0, None, op0=OP.add,
                        )
                    ab[l] = ab_l

        # ---------------- adaln1 apply + projections, interleaved --------
        k1T = act.tile([P, 4, NB], bf16, tag="tB")
        v1 = act.tile([P, 16, H, HD + 1], bf16, tag="tC")
        q1pad = act.tile([P, 4, 2, ROWS], bf16, tag="tD")
        nc.gpsimd.memset(v1[:, :, :, HD:HD + 1], 1.0)
        nc.gpsimd.memset(q1pad, 0.0)
        with (
            tc.tile_pool(name="n1_xn", bufs=3) as xn_pool,
            tc.tile_pool(name="n1_pst", bufs=2, space="PSUM") as pst_pool,
            tc.tile_pool(name="ps_proj1", bufs=4, space="PSUM") as ps_proj,
        ):
            for jc in range(4):
                for it in range(4 * jc, 4 * jc + 4):
                    _adaln_apply_tile(nc, xn_pool, pst_pool, x_tiles[it], it,
                                      ab[1], rstd1, nmr1, h1T, ident_bf16,
                                      on_act=True)
                for dt_ in range(4):
                    ps = ps_proj.tile([P, 512], f32, tag="proj")
                    for kt in range(4):
                        nc.tensor.matmul(
                            ps,
                            a_sb[1, "k"][:, kt, dt_ * P:(dt_ + 1) * P],
                            h1T[:, kt, jc * 512:(jc + 1) * 512],
                            start=(kt == 0), stop=(kt == 3),
                        )
                    nc.vector.tensor_copy(
                        k1T[:, dt_, jc * 512:(jc + 1) * 512], ps
                    )
                for jt in range(4 * jc, 4 * jc + 4):
                    ps = ps_proj.tile([P, 512], f32, tag="proj")
                    for kt in range(4):
                        nc.tensor.matmul(
                            ps,
                            h1T[:, kt, jt * P:(jt + 1) * P],
                            a_sb[1, "v"][:, kt, :],
                            start=(kt == 0), stop=(kt == 3),
                        )
                    nc.vector.tensor_copy(
                        v1[:, jt, :, 0:HD], ps.rearrange("p (h d) -> p h d", h=H)
                    )
                if jc == 0:
                    for dt_ in range(4):
                        ps = ps_proj.tile([P, 512], f32, tag="proj")
                        for kt in range(4):
                            nc.tensor.matmul(
                                ps,
                                a_sb[1, "q"][:, kt, dt_ * P:(dt_ + 1) * P],
                                h1T[:, kt, 0:ROWS],
                                start=(kt == 0), stop=(kt == 3),
                            )
                        nc.vector.tensor_copy(q1pad[0:HD, dt_, 0, :], ps[0:HD, :])
                        nc.vector.tensor_copy(q1pad[HD:P, dt_, 1, :], ps[HD:P, :])

        # ---------------- attention 1 ------------------------------------
        x2 = act.tile([P, 4, D], f32, tag="tF")
        _attention(nc, tc, act, q1pad, k1T, v1, 16, a_sb[1, "o"],
                   a_sb[1, "ob"], ones_row, own_x, x2, "att1")

        # ------- cross-attn prep: k2T, v2 (independent of x) -------------
        v2 = act.tile([P, 8, H, HD + 1], bf16, tag="tI")
        nc.gpsimd.memset(v2[:, :, :, HD:HD + 1], 1.0)
        with tc.tile_pool(name="ps_proj2a", bufs=4, space="PSUM") as ps_proj:
            for dt_ in range(4):
                for jc in range(2):
                    ps = ps_proj.tile([P, 512], f32, tag="proj")
                    for kt in range(4):
                        nc.tensor.matmul(
                            ps,
                            a_sb[2, "k"][:, kt, dt_ * P:(dt_ + 1) * P],
                            condT[:, kt, jc * 512:(jc + 1) * 512],
                            start=(kt == 0), stop=(kt == 3),
                        )
                    nc.vector.tensor_copy(
                        k2T[:, dt_, jc * 512:(jc + 1) * 512], ps
                    )
            for jt in range(8):
                ps = ps_proj.tile([P, 512], f32, tag="proj")
                for kt in range(4):
                    nc.tensor.matmul(
                        ps,
                        condT[:, kt, jt * P:(jt + 1) * P],
                        a_sb[2, "v"][:, kt, :],
                        start=(kt == 0), stop=(kt == 3),
                    )
                nc.vector.tensor_copy(
                    v2[:, jt, :, 0:HD], ps.rearrange("p (h d) -> p h d", h=H)
                )

        # ---------------- adaln2 + cross-attn ----------------------------
        h2T = act.tile([P, 4, ROWS], bf16, tag="tH")
        _adaln_to_hT(nc, tc, lambda it: x2[:, it, :], 4, ab[2], h2T,
                     ident_bf16, eps_sb, "n2")

        q2pad = act.tile([P, 4, 2, ROWS], bf16, tag="tD")
        nc.gpsimd.memset(q2pad, 0.0)
        with tc.tile_pool(name="ps_proj2b", bufs=2, space="PSUM") as ps_proj:
            for dt_ in range(4):
                ps = ps_proj.tile([P, 512], f32, tag="proj")
                for kt in range(4):
                    nc.tensor.matmul(
                        ps,
                        a_sb[2, "q"][:, kt, dt_ * P:(dt_ + 1) * P],
                        h2T[:, kt, :],
                        start=(kt == 0), stop=(kt == 3),
                    )
                nc.vector.tensor_copy(q2pad[0:HD, dt_, 0, :], ps[0:HD, :])
                nc.vector.tensor_copy(q2pad[HD:P, dt_, 1, :], ps[HD:P, :])

        x3 = act.tile([P, 4, D], f32, tag="tG")
        _attention(nc, tc, act, q2pad, k2T, v2, 8, a_sb[2, "o"],
                   a_sb[2, "ob"], ones_row, x2, x3, "att2")

        # ---------------- adaln3 + GEGLU FFN -----------------------------
        h3T = act.tile([P, 4, ROWS], bf16, tag="tJ")
        _adaln_to_hT(nc, tc, lambda it: x3[:, it, :], 4, ab[4], h3T,
                     ident_bf16, eps_sb, "n4")

        # per-ut pipeline: zu/zg -> gelu/stt -> 4 y-accumulator matmuls.
        # y accumulates in 4 persistent PSUM banks across all 16 ut chunks.
        ugT = act.tile([P, 16, ROWS], bf16, tag="tA")
        out_sb = act.tile([P, 4, D], f32, tag="tC")
        with (
            tc.tile_pool(name="ps_z", bufs=4, space="PSUM") as ps_z,
            tc.tile_pool(name="ps_y", bufs=1, space="PSUM") as ps_y,
            tc.tile_pool(name="gact", bufs=3) as gact_pool,
        ):
            y_ps = ps_y.tile([P, 4, D], f32)
            for ut in range(16):
                zu = ps_z.tile([P, ROWS], f32, tag="z")
                zg = ps_z.tile([P, ROWS], f32, tag="z")
                for kt in range(4):
                    nc.tensor.matmul(
                        zu, w1_sb[:, kt, ut * P:(ut + 1) * P],
                        h3T[:, kt, :], start=(kt == 0), stop=(kt == 3),
                    )
                for kt in range(4):
                    nc.tensor.matmul(
                        zg, w1_sb[:, kt, (16 + ut) * P:(17 + ut) * P],
                        h3T[:, kt, :], start=(kt == 0), stop=(kt == 3),
                    )
                gact = gact_pool.tile([P, ROWS], bf16, tag="gact")
                nc.scalar.activation(
                    gact, zg, AF.Gelu, bias=b1_sb[:, 16 + ut:17 + ut], scale=1.0
                )
                nc.vector.scalar_tensor_tensor(
                    ugT[:, ut, :], zu, b1_sb[:, ut:ut + 1], gact,
                    op0=OP.add, op1=OP.mult,
                )
                for it in range(4):
                    nc.tensor.matmul(
                        y_ps[:, it, :], ugT[:, ut, it * P:(it + 1) * P],
                        w2_sb[:, ut, :],
                        start=(ut == 0), stop=False,
                    )
            for it in range(4):
                nc.tensor.matmul(
                    y_ps[:, it, :], ones_row[0:1, 0:P], b2_row,
                    start=False, stop=True,
                )
                nc.vector.tensor_tensor(
                    out_sb[:, it, :], y_ps[:, it, :], x3[:, it, :], op=OP.add
                )

        for it_ in range(4):
            nc.sync.dma_start(out[:][it_ * P:(it_ + 1) * P, :], out_sb[:, it_, :])

    nc.compile()
    return nc


def _prep_shared(inputs):
    """Pre-cast weights to bf16 and pre-arrange into SBUF layouts (host-side
    layout prep, shared by all cores)."""
    bf = ml_dtypes.bfloat16

    def pkn(w, ktiles):
        # [ktiles*128, n] f32 -> [128, ktiles, n] bf16
        n = w.shape[1]
        return np.ascontiguousarray(
            w.reshape(ktiles, P, n).transpose(1, 0, 2).astype(bf))

    shared = {}
    for l in (1, 2, 4):
        shared[f"nw{l}"] = pkn(np.asarray(inputs[f"n{l}_w"], np.float32), 4)
        shared[f"nb{l}"] = np.ascontiguousarray(inputs[f"n{l}_b"], np.float32)
    for a in (1, 2):
        ws = [pkn(np.asarray(inputs[f"a{a}_{w}"], np.float32), 4)
              for w in "qkvo"]
        shared[f"a{a}s"] = np.ascontiguousarray(np.stack(ws, axis=2))
        shared[f"a{a}ob"] = np.asarray(inputs[f"a{a}_ob"], np.float32).astype(bf)
    shared["ffw1"] = pkn(np.asarray(inputs["ff_w1"], np.float32), 4)
    shared["ffw2"] = pkn(np.asarray(inputs["ff_w2"], np.float32), 16)
    shared["ffb1"] = np.ascontiguousarray(
        np.asarray(inputs["ff_b1"], np.float32).reshape(32, P).T)
    shared["ffb2"] = np.asarray(inputs["ff_b2"], np.float32).astype(bf)
    return shared


def _shard_inputs(inputs):
    """Build the 8 per-core input maps."""
    bf = ml_dtypes.bfloat16
    x = np.ascontiguousarray(inputs["x"], dtype=np.float32)
    t = np.ascontiguousarray(inputs["t"], dtype=np.float32)
    cond = np.ascontiguousarray(inputs["cond"], dtype=np.float32)
    shared = _prep_shared(inputs)
    per_batch = {}
    for b in range(B):
        condT = cond[b].T.reshape(4, P, NCTX).transpose(1, 0, 2)
        tT = t[b, 0].reshape(4, P).T
        per_batch[b] = (
            np.ascontiguousarray(condT.astype(bf)),
            np.ascontiguousarray(tT.astype(bf)),
        )
    in_maps = []
    for c in range(NCORES):
        b = c // 4
        r0 = (c % 4) * ROWS
        m = dict(shared)
        m["xb"] = np.ascontiguousarray(np.roll(x[b], -r0, axis=0))
        m["condT"], m["tT"] = per_batch[b]
        in_maps.append(m)
    return in_maps


def kernel(**inputs) -> np.ndarray:
    if "nc" not in _CACHED:
        _CACHED["nc"] = build()
    nc = _CACHED["nc"]
    in_maps = _shard_inputs(inputs)
    res = run_bass_kernel_spmd(nc, in_maps, core_ids=list(range(NCORES)))
    outs = [res.results[c]["out"] for c in range(NCORES)]
    full = np.concatenate(outs, axis=0).reshape(B, N, D)
    return full.astype(np.float32)


# revision 33
# speedup vs baseline: 1.5701x; 1.0783x over previous
"""BasicTransformerBlock on 8 TRN2 NeuronCores.

Sharding: sequence-parallel, zero collectives. The [B=2, N=2048, D=512]
residual stream is split into 8 row-blocks of 512 (4 cores per batch
element). Every core recomputes the cheap batch-wide work it needs
(adaln1 + K/V projections over its batch's 2048 rows, cond K/V), and does
attention / FFN only for its own 512 query rows.

Host-side prep (part of kernel()'s sharding layer, shared across cores):
weights are pre-cast to bf16 and pre-rearranged into the [partition, ktile,
cols] SBUF layouts, and cond is pre-transposed, so every DMA is a fast
contiguous HWDGE transfer (no software-DGE casting scatter loads) and no
on-chip cond transposes are needed. x stays f32 (layernorm stats need it).
Per-core x is pre-rotated with np.roll so "own" rows are always rows 0:512;
attention is permutation-invariant over keys, so rolled K/V is fine.

Attention: transposed scores sT[j, i] so exp() runs on ScalarE straight out
of PSUM. Score matmuls are DENSE (K=128): both heads of a pair are stacked
in the stationary operand and the query operand is zero-padded per head.
The PE_HAM activity monitor only grants the 2.4 GHz clock to full-array
matmuls; K=64 attention matmuls would run the whole phase at 1.2 GHz.
The 65th v-column of ones makes the softmax denominator fall out of the
attn@v matmul.

PSUM during attention: a 6-slot score ring (6 banks) + 2 accumulator
banks. exp is issued per 3-slot granule ([128, 3, 512] per ACTIVATE) to
amortize ACT instruction overhead, and the loop is skewed exp(g-1) /
scores(g) / av(g-2) so the PE batch for step g only depends on ACT work
from step g-2 -- the engines overlap instead of ping-ponging.
"""

import contextlib

import ml_dtypes
import numpy as np

import concourse.bass as bass
import concourse.mybir as mybir
import concourse.tile as tile
from concourse import bacc
from concourse.bass_utils import run_bass_kernel_spmd
from concourse.masks import make_identity

dt = mybir.dt
AF = mybir.ActivationFunctionType
OP = mybir.AluOpType

B, N, D = 2, 2048, 512
NCTX = 1024          # cond length
H = 8                # heads
HD = D // H          # 64
EPS = 1e-5
P = 128              # partitions
NCORES = 8
ROWS = 512           # own rows per core
NB = N               # batch rows per core (2048)
SCALE = HD ** -0.5   # 0.125

f32 = dt.float32
bf16 = dt.bfloat16

_CACHED = {}


def _adaln_stats(nc, stat_pool, src_tiles, n_tiles, eps_sb, chunk=4):
    """bn_stats/aggr + rstd/nmr for n_tiles row-tiles. Returns (rstd_all, nmr_all)."""
    mv_all = stat_pool.tile([P, n_tiles, 2], f32)
    rstd_all = stat_pool.tile([P, n_tiles], f32)
    nmr_all = stat_pool.tile([P, n_tiles], f32)
    for c0 in range(0, n_tiles, chunk):
        for it in range(c0, c0 + chunk):
            stats = stat_pool.tile([P, 6], f32, tag="stats")
            nc.vector.bn_stats(stats, src_tiles(it))
            nc.vector.bn_aggr(mv_all[:, it, :], stats)
        cs = slice(c0, c0 + chunk)
        nc.scalar.activation(rstd_all[:, cs], mv_all[:, cs, 1], AF.Sqrt,
                             bias=eps_sb, scale=1.0)
        nc.vector.reciprocal(rstd_all[:, cs], rstd_all[:, cs])
        nc.vector.scalar_tensor_tensor(
            nmr_all[:, cs], mv_all[:, cs, 0], -1.0, rstd_all[:, cs],
            op0=OP.mult, op1=OP.mult,
        )
    return rstd_all, nmr_all


def _adaln_apply_tile(nc, xn_pool, pst_pool, src, it, ab, rstd_all, nmr_all,
                      hT, ident_bf16, on_act=False):
    """One tile: xn = (x-mean)*rstd -> PE transpose -> (1+scale)/shift -> hT."""
    xn = xn_pool.tile([P, 512], bf16, tag="xn")
    if on_act:
        nc.scalar.activation(xn, src, AF.Identity,
                             bias=nmr_all[:, it:it + 1],
                             scale=rstd_all[:, it:it + 1])
    else:
        nc.vector.tensor_scalar(
            xn, src, rstd_all[:, it:it + 1], nmr_all[:, it:it + 1],
            op0=OP.mult, op1=OP.add,
        )
    xnt = pst_pool.tile([P, 4, P], bf16, tag="xnt")
    for b in range(4):
        nc.tensor.transpose(xnt[:, b, :], xn[:, b * P:(b + 1) * P], ident_bf16)
    for b in range(4):
        nc.vector.tensor_scalar(
            hT[:, b, it * P:(it + 1) * P], xnt[:, b, :],
            ab[:, b:b + 1], ab[:, 4 + b:5 + b],
            op0=OP.mult, op1=OP.add,
        )


def _adaln_to_hT(nc, tc, src_tiles, n_tiles, ab, hT, ident_bf16, eps_sb, name):
    with contextlib.ExitStack() as actx:
        stat_pool = actx.enter_context(tc.tile_pool(name=f"{name}_stat", bufs=4))
        xn_pool = actx.enter_context(tc.tile_pool(name=f"{name}_xn", bufs=3))
        pst_pool = actx.enter_context(
            tc.tile_pool(name=f"{name}_pst", bufs=2, space="PSUM"))
        rstd, nmr = _adaln_stats(nc, stat_pool, src_tiles, n_tiles, eps_sb,
                                 chunk=n_tiles)
        for it in range(n_tiles):
            _adaln_apply_tile(nc, xn_pool, pst_pool, src_tiles(it), it, ab,
                              rstd, nmr, hT, ident_bf16, on_act=False)


def _attention(nc, tc, act, qpad, kT, v, njt, wo, ob_row, ones_row,
               x_res, x_out, name):
    """Dense-score attention for 8 heads (4 pairs) over own 512 rows.

    qpad: [128, 4, 2, ROWS] bf16 zero-padded per head half.
    kT:   [128, 4, njt*128] bf16 (partitions = paired head dims).
    v:    [128, njt, 8, 65] bf16 (col 64 of each head = 1.0).
    Writes x_out = attn_out @ wo + ob + x_res  (all [128, 4, 512] f32).
    """
    av_all = act.tile([P, 4, ROWS], bf16, tag="tH")
    S = 2 * njt                       # score slots per pair
    G = (S + 2) // 3                  # exp granules per pair
    with (
        tc.tile_pool(name=f"{name}_ps_s", bufs=2, space="PSUM") as ps_s,
        tc.tile_pool(name=f"{name}_ps_av", bufs=1, space="PSUM") as ps_av,
        tc.tile_pool(name=f"{name}_et", bufs=3) as et_pool,
        tc.tile_pool(name=f"{name}_dn", bufs=4) as dn_pool,
        tc.tile_pool(name=f"{name}_rb", bufs=2) as rb_pool,
    ):
        def pair_tail(pht, pavp):
            # softmax denominators: row 64 of each accumulator. Broadcast
            # across partitions with K=1 matmuls into a score-pool tile
            # (keeps the ping-pong rotation in phase), then one fast
            # reciprocal for the pair. Emitted at the next pair's start so
            # the PE never drains at pair boundaries (a >3.4us PE gap
            # re-throttles the HAM clock).
            rbt = ps_s.tile([P, 3, ROWS], f32, tag="sgr", name="rbt")
            rb_ps = rbt[:, 0, :]
            for hl in range(2):
                dnm = dn_pool.tile([1, ROWS], bf16, tag="dnm")
                nc.vector.tensor_copy(dnm, pavp[HD:HD + 1, hl, :])
                nc.tensor.matmul(
                    rb_ps[hl * HD:(hl + 1) * HD, :],
                    ones_row[0:1, 0:HD], dnm,
                    start=True, stop=True,
                )
            rb_sb = rb_pool.tile([P, ROWS], f32, tag="rb")
            nc.vector.reciprocal_approx_fast(rb_sb, rb_ps)
            for hl in range(2):
                po = hl * HD
                nc.vector.scalar_tensor_tensor(
                    av_all[po:po + HD, pht, :],
                    pavp[0:HD, hl, :], 1.0, rb_sb[po:po + HD, :],
                    op0=OP.mult, op1=OP.mult,
                )

        prev = None
        for ht in range(4):           # head pair (2ht, 2ht+1)
            if prev is not None:
                pair_tail(*prev)
                prev = None
            avp = ps_av.tile([P, 2, ROWS], f32, tag="av")
            ets = {}
            sgr = {}
            for g in range(G + 2):
                if 1 <= g <= G:
                    lo, hi = 3 * (g - 1), min(3 * (g - 1) + 3, S)
                    ng = hi - lo
                    et = et_pool.tile([P, 3, ROWS], bf16, tag="et")
                    ets[g - 1] = (et, lo, hi)
                    nc.scalar.activation(
                        et[:, 0:ng, :], sgr[g - 1][:, 0:ng, :], AF.Exp,
                        scale=SCALE,
                    )
                if g < G:
                    lo, hi = 3 * g, min(3 * g + 3, S)
                    sg = ps_s.tile([P, 3, ROWS], f32, tag="sgr")
                    sgr[g] = sg
                    for s in range(lo, hi):
                        jt, hl = s // 2, s % 2
                        nc.tensor.matmul(
                            sg[:, s - lo, :],
                            kT[:, ht, jt * P:(jt + 1) * P],
                            qpad[:, ht, hl, :],
                            start=True, stop=True,
                        )
                if g >= 2:
                    et, lo, hi = ets.pop(g - 2)
                    sgr.pop(g - 2, None)
                    for i, s in enumerate(range(lo, hi)):
                        jt, hl = s // 2, s % 2
                        nc.tensor.matmul(
                            avp[0:HD + 1, hl, :], v[:, jt, 2 * ht + hl, :],
                            et[:, i, :],
                            start=(jt == 0), stop=(jt == njt - 1),
                        )
            prev = (ht, avp)
        pair_tail(*prev)
    # out-projection + bias + residual
    with tc.tile_pool(name=f"{name}_ps_o", bufs=2, space="PSUM") as ps_o:
        for it in range(4):
            ps = ps_o.tile([P, D], f32, tag="o")
            for dt_ in range(4):
                nc.tensor.matmul(
                    ps, av_all[:, dt_, it * P:(it + 1) * P], wo[:, dt_, :],
                    start=(dt_ == 0), stop=False,
                )
            nc.tensor.matmul(
                ps, ones_row[0:1, 0:P], ob_row, start=False, stop=True,
            )
            nc.vector.tensor_tensor(x_out[:, it, :], ps, x_res[:, it, :], op=OP.add)


def build():
    nc = bacc.Bacc(None, target_bir_lowering=False)

    # -------- I/O (weights arrive pre-cast/pre-arranged from the host) ----
    xb = nc.dram_tensor("xb", [NB, D], f32, kind="ExternalInput")
    condT_d = nc.dram_tensor("condT", [P, 4, NCTX], bf16, kind="ExternalInput")
    tT_d = nc.dram_tensor("tT", [P, 4], bf16, kind="ExternalInput")
    nw_d = {}
    nb_d = {}
    for l in (1, 2, 4):
        nw_d[l] = nc.dram_tensor(f"nw{l}", [P, 4, 2 * D], bf16,
                                 kind="ExternalInput")
        nb_d[l] = nc.dram_tensor(f"nb{l}", [2 * D], f32, kind="ExternalInput")
    as_d = {}
    aob_d = {}
    for a in (1, 2):
        as_d[a] = nc.dram_tensor(f"a{a}s", [P, 4, 4, D], bf16,
                                 kind="ExternalInput")
        aob_d[a] = nc.dram_tensor(f"a{a}ob", [D], bf16, kind="ExternalInput")
    ffw1_d = nc.dram_tensor("ffw1", [P, 4, 8 * D], bf16, kind="ExternalInput")
    ffw2_d = nc.dram_tensor("ffw2", [P, 16, D], bf16, kind="ExternalInput")
    ffb1_d = nc.dram_tensor("ffb1", [P, 32], f32, kind="ExternalInput")
    ffb2_d = nc.dram_tensor("ffb2", [D], bf16, kind="ExternalInput")
    out = nc.dram_tensor("out", [ROWS, D], f32, kind="ExternalOutput")

    with tile.TileContext(nc) as tc, contextlib.ExitStack() as ctx:
        const = ctx.enter_context(tc.tile_pool(name="const", bufs=1))
        wpool = ctx.enter_context(tc.tile_pool(name="wpool", bufs=1))
        act = ctx.enter_context(tc.tile_pool(name="act", bufs=1))
        xr_pool = ctx.enter_context(tc.tile_pool(name="xrp", bufs=6))
        n1_stat = ctx.enter_context(tc.tile_pool(name="n1_stat", bufs=4))

        ident_bf16 = const.tile([P, P], bf16)
        make_identity(nc, ident_bf16)
        ident_f32 = const.tile([P, P], f32)
        make_identity(nc, ident_f32)
        ones_row = const.tile([1, P], bf16)
        nc.vector.memset(ones_row, 1.0)
        eps_sb = const.tile([P, 1], f32)
        nc.vector.memset(eps_sb, EPS)

        # ---------------- DMA issue order --------------------------------
        # qACT: weights in first-use order.  qSP: x, small rows, ff tail.
        tT = const.tile([P, 4], bf16)
        nc.scalar.dma_start(tT, tT_d[:])
        ab = {}
        with (
            tc.tile_pool(name="nwp", bufs=1) as nwp,
            tc.tile_pool(name="embp", bufs=1) as embp,
        ):
            nw_sb = {}
            for l in (1, 2):
                nw_sb[l] = nwp.tile([P, 4, 2 * D], bf16, tag=f"nw{l}",
                                    name=f"nw_sb{l}")
                nc.scalar.dma_start(nw_sb[l], nw_d[l][:])

            a_sb = {}
            stacks = {}
            for a, wtag in ((1, "wbig1"), (2, "wbig2")):
                stack = wpool.tile([P, 4, 4, D], bf16, tag=wtag,
                                   name=f"a{a}stk")
                stacks[a] = stack
                for wi, w in enumerate("qkvo"):
                    a_sb[a, w] = stack[:, :, wi, :]
            pass
            for a in (1, 2):
                ob = wpool.tile([1, D], bf16, tag=f"a{a}ob", name=f"a{a}ob_sb")
                a_sb[a, "ob"] = ob

            h1T = act.tile([P, 4, NB], bf16, tag="tA")
            own_x = act.tile([P, 4, D], f32, tag="tE")
            x_tiles = {}
            for it in range(16):
                if it < 4:
                    dst = own_x[:, it, :]
                else:
                    dst = xr_pool.tile([P, D], f32, tag="xr", name=f"xr{it}")
                nc.gpsimd.dma_start(dst, xb[:][it * P:(it + 1) * P, :])
                x_tiles[it] = dst

            # condT + k2T share the tX region
            ctk2 = act.tile([P, 2, 4, NCTX], bf16, tag="tX")
            condT = ctk2[:, 0, :, :]
            k2T = ctk2[:, 1, :, :]
            # big weights + condT on the otherwise-idle SWDGE queue: HWDGE
            # trigger instructions cost ~2.5us EACH on their engine's queue
            # and were starving the ACT stats chain.
            nc.gpsimd.dma_start(stacks[1], as_d[1][:])
            nc.gpsimd.dma_start(a_sb[1, "ob"],
                                aob_d[1][:].rearrange("(a n) -> a n", a=1))
            nc.gpsimd.dma_start(stacks[2], as_d[2][:])
            nc.gpsimd.dma_start(a_sb[2, "ob"],
                                aob_d[2][:].rearrange("(a n) -> a n", a=1))
            nc.gpsimd.dma_start(condT, condT_d[:])
            nw_sb[4] = nwp.tile([P, 4, 2 * D], bf16, tag="nw1", name="nw_sb4")
            nc.gpsimd.dma_start(nw_sb[4], nw_d[4][:])

            nb_row = {}
            for l in (1, 2, 4):
                nb_row[l] = embp.tile([1, 2 * D], f32, tag="nbrow",
                                      name=f"nb_row{l}")
                nc.sync.dma_start(nb_row[l],
                                  nb_d[l][:].rearrange("(a n) -> a n", a=1))
            b1_sb = const.tile([P, 32], f32)
            nc.sync.dma_start(b1_sb, ffb1_d[:])
            b2_row = const.tile([1, D], bf16)
            nc.sync.dma_start(b2_row, ffb2_d[:].rearrange("(a n) -> a n", a=1))
            # ff weights on qSP after x: landed long before the FFN needs
            # them, but the buffers alias a1s/a2s so they wait on attention.
            w1_sb = wpool.tile([P, 4, 8 * D], bf16, tag="wbig1")
            nc.sync.dma_start(w1_sb, ffw1_d[:])
            w2_sb = wpool.tile([P, 16, D], bf16, tag="wbig2")
            nc.sync.dma_start(w2_sb, ffw2_d[:])

            # PE warmup: dependency-free matmuls fill the startup DMA window
            with tc.tile_pool(name="warm", bufs=1, space="PSUM") as warm_pool:
                wps = warm_pool.tile([P, P], f32)
                for _ in range(30):
                    nc.tensor.matmul(wps, ident_bf16, ident_bf16,
                                     start=True, stop=True)

            # ------------- norm scale/shift params -----------------------
            # only ab[1] gates the adaln1 loop; l=2,4 run after it so their
            # nw DMA arrival never stalls the PE stream.
            def emb_ab(l, ps_emb):
                emb_ps = ps_emb.tile([1, 2 * D], f32, tag="embps",
                                     name=f"emb_ps{l}")
                for half in range(2):
                    for kt in range(4):
                        nc.tensor.matmul(
                            emb_ps[:, half * D:(half + 1) * D],
                            tT[:, kt:kt + 1],
                            nw_sb[l][:, kt, half * D:(half + 1) * D],
                            start=(kt == 0), stop=(kt == 3),
                        )
                emb_row = embp.tile([1, 2 * D], f32, tag="embrow",
                                    name=f"emb_row{l}")
                nc.vector.tensor_tensor(emb_row, emb_ps, nb_row[l], op=OP.add)
                ab_l = const.tile([P, 8], f32, tag=f"ab{l}", name=f"ab_{l}")
                for col in range(8):
                    tp = ps_emb.tile([P, 1], f32, tag="embT")
                    nc.tensor.transpose(
                        tp, emb_row[0:1, col * P:(col + 1) * P],
                        ident_f32[0:1, 0:1]
                    )
                    nc.vector.tensor_scalar(
                        ab_l[:, col:col + 1], tp,
                        1.0 if col < 4 else 0.0, None, op0=OP.add,
                    )
                ab[l] = ab_l

            with tc.tile_pool(name="ps_emb1", bufs=2, space="PSUM") as pe1:
                emb_ab(1, pe1)

            # --------- adaln1 apply + projections, interleaved -----------
            # Per 512-row block: stats chunk -> xn (ACT) -> PE transpose ->
            # affine (DVE) -> dense k1/v1/q matmuls, with the PSUM->SBUF
            # drains on the otherwise-idle ACT engine.
            k1T = act.tile([P, 4, NB], bf16, tag="tB")
            v1 = act.tile([P, 16, H, HD + 1], bf16, tag="tC")
            q1pad = act.tile([P, 4, 2, ROWS], bf16, tag="tD")
            nc.gpsimd.memset(v1[:, :, :, HD:HD + 1], 1.0)
            nc.gpsimd.memset(q1pad, 0.0)
            mv1 = n1_stat.tile([P, 16, 2], f32)
            rstd1 = n1_stat.tile([P, 16], f32)
            nmr1 = n1_stat.tile([P, 16], f32)
            with (
                tc.tile_pool(name="n1_xn", bufs=3) as xn_pool,
                tc.tile_pool(name="n1_pst", bufs=2, space="PSUM") as pst_pool,
                tc.tile_pool(name="ps_proj1", bufs=4, space="PSUM") as ps_proj,
            ):
                for jc in range(4):
                    cs = slice(4 * jc, 4 * jc + 4)
                    for it in range(4 * jc, 4 * jc + 4):
                        stats = n1_stat.tile([P, 6], f32, tag="stats")
                        nc.vector.bn_stats(stats, x_tiles[it])
                        nc.vector.bn_aggr(mv1[:, it, :], stats)
                    nc.scalar.activation(rstd1[:, cs], mv1[:, cs, 1], AF.Sqrt,
                                         bias=eps_sb, scale=1.0)
                    nc.vector.reciprocal(rstd1[:, cs], rstd1[:, cs])
                    nc.vector.scalar_tensor_tensor(
                        nmr1[:, cs], mv1[:, cs, 0], -1.0, rstd1[:, cs],
                        op0=OP.mult, op1=OP.mult,
                    )
                    for it in range(4 * jc, 4 * jc + 4):
                        _adaln_apply_tile(nc, xn_pool, pst_pool, x_tiles[it],
                                          it, ab[1], rstd1, nmr1, h1T,
                                          ident_bf16, on_act=True)
                    for dt_ in range(4):
                        ps = ps_proj.tile([P, 512], f32, tag="proj")
                        for kt in range(4):
                            nc.tensor.matmul(
                                ps,
                                a_sb[1, "k"][:, kt, dt_ * P:(dt_ + 1) * P],
                                h1T[:, kt, jc * 512:(jc + 1) * 512],
                                start=(kt == 0), stop=(kt == 3),
                            )
                        nc.scalar.activation(
                            k1T[:, dt_, jc * 512:(jc + 1) * 512], ps, AF.Copy
                        )
                    for jt in range(4 * jc, 4 * jc + 4):
                        ps = ps_proj.tile([P, 512], f32, tag="proj")
                        for kt in range(4):
                            nc.tensor.matmul(
                                ps,
                                h1T[:, kt, jt * P:(jt + 1) * P],
                                a_sb[1, "v"][:, kt, :],
                                start=(kt == 0), stop=(kt == 3),
                            )
                        nc.scalar.activation(
                            v1[:, jt, :, 0:HD],
                            ps.rearrange("p (h d) -> p h d", h=H), AF.Copy
                        )
                    if jc == 0:
                        for dt_ in range(4):
                            ps = ps_proj.tile([P, 512], f32, tag="proj")
                            for kt in range(4):
                                nc.tensor.matmul(
                                    ps,
                                    a_sb[1, "q"][:, kt, dt_ * P:(dt_ + 1) * P],
                                    h1T[:, kt, 0:ROWS],
                                    start=(kt == 0), stop=(kt == 3),
                                )
                            nc.scalar.activation(q1pad[0:HD, dt_, 0, :],
                                                 ps[0:HD, :], AF.Copy)
                            nc.scalar.activation(q1pad[HD:P, dt_, 1, :],
                                                 ps[HD:P, :], AF.Copy)

            with tc.tile_pool(name="ps_emb2", bufs=2, space="PSUM") as pe2:
                emb_ab(2, pe2)
                emb_ab(4, pe2)

        # ---------------- attention 1 ------------------------------------
        x2 = act.tile([P, 4, D], f32, tag="tF")
        _attention(nc, tc, act, q1pad, k1T, v1, 16, a_sb[1, "o"],
                   a_sb[1, "ob"], ones_row, own_x, x2, "att1")

        # ------- cross-attn prep: k2T, v2 (independent of x) -------------
        v2 = act.tile([P, 8, H, HD + 1], bf16, tag="tI")
        nc.gpsimd.memset(v2[:, :, :, HD:HD + 1], 1.0)
        with tc.tile_pool(name="ps_proj2a", bufs=4, space="PSUM") as ps_proj:
            for dt_ in range(4):
                for jc in range(2):
                    ps = ps_proj.tile([P, 512], f32, tag="proj")
                    for kt in range(4):
                        nc.tensor.matmul(
                            ps,
                            a_sb[2, "k"][:, kt, dt_ * P:(dt_ + 1) * P],
                            condT[:, kt, jc * 512:(jc + 1) * 512],
                            start=(kt == 0), stop=(kt == 3),
                        )
                    nc.vector.tensor_copy(
                        k2T[:, dt_, jc * 512:(jc + 1) * 512], ps
                    )
            for jt in range(8):
                ps = ps_proj.tile([P, 512], f32, tag="proj")
                for kt in range(4):
                    nc.tensor.matmul(
                        ps,
                        condT[:, kt, jt * P:(jt + 1) * P],
                        a_sb[2, "v"][:, kt, :],
                        start=(kt == 0), stop=(kt == 3),
                    )
                nc.vector.tensor_copy(
                    v2[:, jt, :, 0:HD], ps.rearrange("p (h d) -> p h d", h=H)
                )

        # ---------------- adaln2 + cross-attn ----------------------------
        h2T = act.tile([P, 4, ROWS], bf16, tag="tH")
        _adaln_to_hT(nc, tc, lambda it: x2[:, it, :], 4, ab[2], h2T,
                     ident_bf16, eps_sb, "n2")

        q2pad = act.tile([P, 4, 2, ROWS], bf16, tag="tD")
        nc.gpsimd.memset(q2pad, 0.0)
        with tc.tile_pool(name="ps_proj2b", bufs=2, space="PSUM") as ps_proj:
            for dt_ in range(4):
                ps = ps_proj.tile([P, 512], f32, tag="proj")
                for kt in range(4):
                    nc.tensor.matmul(
                        ps,
                        a_sb[2, "q"][:, kt, dt_ * P:(dt_ + 1) * P],
                        h2T[:, kt, :],
                        start=(kt == 0), stop=(kt == 3),
                    )
                nc.vector.tensor_copy(q2pad[0:HD, dt_, 0, :], ps[0:HD, :])
                nc.vector.tensor_copy(q2pad[HD:P, dt_, 1, :], ps[HD:P, :])

        x3 = act.tile([P, 4, D], f32, tag="tG")
        _attention(nc, tc, act, q2pad, k2T, v2, 8, a_sb[2, "o"],
                   a_sb[2, "ob"], ones_row, x2, x3, "att2")

        # ---------------- adaln3 + GEGLU FFN -----------------------------
        h3T = act.tile([P, 4, ROWS], bf16, tag="tJ")
        _adaln_to_hT(nc, tc, lambda it: x3[:, it, :], 4, ab[4], h3T,
                     ident_bf16, eps_sb, "n4")

        # per-ut pipeline: zu/zg -> gelu/stt -> 4 y-accumulator matmuls.
        # y accumulates in 4 persistent PSUM banks across all 16 ut chunks.
        ugT = act.tile([P, 16, ROWS], bf16, tag="tA")
        out_sb = act.tile([P, 4, D], f32, tag="tC")
        with (
            tc.tile_pool(name="ps_z", bufs=4, space="PSUM") as ps_z,
            tc.tile_pool(name="ps_y", bufs=1, space="PSUM") as ps_y,
            tc.tile_pool(name="gact", bufs=3) as gact_pool,
        ):
            y_ps = ps_y.tile([P, 4, D], f32)
            for ut in range(16):
                zu = ps_z.tile([P, ROWS], f32, tag="z")
                zg = ps_z.tile([P, ROWS], f32, tag="z")
                for kt in range(4):
                    nc.tensor.matmul(
                        zu, w1_sb[:, kt, ut * P:(ut + 1) * P],
                        h3T[:, kt, :], start=(kt == 0), stop=(kt == 3),
                    )
                for kt in range(4):
                    nc.tensor.matmul(
                        zg, w1_sb[:, kt, (16 + ut) * P:(17 + ut) * P],
                        h3T[:, kt, :], start=(kt == 0), stop=(kt == 3),
                    )
                gact = gact_pool.tile([P, ROWS], bf16, tag="gact")
                nc.scalar.activation(
                    gact, zg, AF.Gelu, bias=b1_sb[:, 16 + ut:17 + ut], scale=1.0
                )
                nc.vector.scalar_tensor_tensor(
                    ugT[:, ut, :], zu, b1_sb[:, ut:ut + 1], gact,
                    op0=OP.add, op1=OP.mult,
                )
                for it in range(4):
                    nc.tensor.matmul(
                        y_ps[:, it, :], ugT[:, ut, it * P:(it + 1) * P],
                        w2_sb[:, ut, :],
                        start=(ut == 0), stop=False,
                    )
            for it in range(4):
                nc.tensor.matmul(
                    y_ps[:, it, :], ones_row[0:1, 0:P], b2_row,
                    start=False, stop=True,
                )
                nc.vector.tensor_tensor(
                    out_sb[:, it, :], y_ps[:, it, :], x3[:, it, :], op=OP.add
                )

        for it_ in range(4):
            nc.sync.dma_start(out[:][it_ * P:(it_ + 1) * P, :], out_sb[:, it_, :])

    nc.compile()
    return nc


def _prep_shared(inputs):
    """Pre-cast weights to bf16 and pre-arrange into SBUF layouts (host-side
    layout prep, shared by all cores)."""
    bf = ml_dtypes.bfloat16

    def pkn(w, ktiles):
        # [ktiles*128, n] f32 -> [128, ktiles, n] bf16
        n = w.shape[1]
        return np.ascontiguousarray(
            w.reshape(ktiles, P, n).transpose(1, 0, 2).astype(bf))

    shared = {}
    for l in (1, 2, 4):
        shared[f"nw{l}"] = pkn(np.asarray(inputs[f"n{l}_w"], np.float32), 4)
        shared[f"nb{l}"] = np.ascontiguousarray(inputs[f"n{l}_b"], np.float32)
    for a in (1, 2):
        ws = [pkn(np.asarray(inputs[f"a{a}_{w}"], np.float32), 4)
              for w in "qkvo"]
        shared[f"a{a}s"] = np.ascontiguousarray(np.stack(ws, axis=2))
        shared[f"a{a}ob"] = np.asarray(inputs[f"a{a}_ob"], np.float32).astype(bf)
    shared["ffw1"] = pkn(np.asarray(inputs["ff_w1"], np.float32), 4)
    shared["ffw2"] = pkn(np.asarray(inputs["ff_w2"], np.float32), 16)
    shared["ffb1"] = np.ascontiguousarray(
        np.asarray(inputs["ff_b1"], np.float32).reshape(32, P).T)
    shared["ffb2"] = np.asarray(inputs["ff_b2"], np.float32).astype(bf)
    return shared


def _shard_inputs(inputs):
    """Build the 8 per-core input maps."""
    bf = ml_dtypes.bfloat16
    x = np.ascontiguousarray(inputs["x"], dtype=np.float32)
    t = np.ascontiguousarray(inputs["t"], dtype=np.float32)
    cond = np.ascontiguousarray(inputs["cond"], dtype=np.float32)
    shared = _prep_shared(inputs)
    per_batch = {}
    for b in range(B):
        condT = cond[b].T.reshape(4, P, NCTX).transpose(1, 0, 2)
        tT = t[b, 0].reshape(4, P).T
        per_batch[b] = (
            np.ascontiguousarray(condT.astype(bf)),
            np.ascontiguousarray(tT.astype(bf)),
        )
    in_maps = []
    for c in range(NCORES):
        b = c // 4
        r0 = (c % 4) * ROWS
        m = dict(shared)
        m["xb"] = np.ascontiguousarray(np.roll(x[b], -r0, axis=0))
        m["condT"], m["tT"] = per_batch[b]
        in_maps.append(m)
    return in_maps


def kernel(**inputs) -> np.ndarray:
    if "nc" not in _CACHED:
        _CACHED["nc"] = build()
    nc = _CACHED["nc"]
    in_maps = _shard_inputs(inputs)
    res = run_bass_kernel_spmd(nc, in_maps, core_ids=list(range(NCORES)))
    outs = [res.results[c]["out"] for c in range(NCORES)]
    full = np.concatenate(outs, axis=0).reshape(B, N, D)
    return full.astype(np.float32)
